# revision 22
# baseline (speedup 1.0000x reference)
"""GAT (2-head) + 3x dense/LayerNorm + pairwise-distance kernel for 8 TRN2 NeuronCores.

Strategy: dst-sharded edge processing (one-hot matmul aggregation), replicated
small dense weights, row-block-sharded NxN cdist output.

v3: fp16 htable/gather rows, host-precomputed one-hot tables (no on-device
is_eq), 2-queue GC=12 gathers, whole-x preload, split-fp16 cdist matmuls,
fp16 output.
"""
import sys

import numpy as np

# Environment bootstrap (harness may run from a bare directory).
for _p in ("/root/.axon_site", "/root/.axon_site/_ro/trn_rl_repo",
           "/root/.axon_site/_ro/pypackages", "/opt/trn_rl_repo"):
    if _p not in sys.path:
        sys.path.append(_p)

import concourse.bass as bass
import concourse.bacc as bacc
import concourse.mybir as mybir
import concourse.tile as tile
from concourse.masks import make_identity
from concourse.bass_utils import run_bass_kernel_spmd

dt = mybir.dt
OP = mybir.AluOpType
AF = mybir.ActivationFunctionType

N = 10000
NPAD = 10112          # 79 * 128
NB = 79               # node blocks (phase A)
FIN = 256
F = 128               # per-head GAT dim
H = 2
HROW = 384            # htable row fp16 elems (768B, multiple of 256B)
CORES = 8
SHARD = 1250          # dst nodes per core
RB = 10               # dst blocks per core
RPAD = 1280
CCH = 512             # cdist column chunk
NCOL = 10240          # padded output columns
EPS = 1e-5

_BUILD_CACHE = {}
_LAST_RESULTS = None


GC = 6   # tile groups per dma_gather call (768 idxs = 48 descs/engine)
SUB = 3  # tiles per a_d/exp subchunk


def _build(TB, phases="ABCDE"):
    """Build the 8-core SPMD program. TB = gather tile groups per dst block (mult of GC)."""
    assert TB % GC == 0
    NCH = TB // GC
    S16 = 8 * TB          # idx columns ([128, S16] wrapped int16)

    nc = bacc.Bacc("TRN2", target_bir_lowering=False, debug=False,
                   num_devices=CORES, num_swdge_queues=2)

    def din(name, shape, d=dt.float32):
        return nc.dram_tensor(name, shape, d, kind="ExternalInput").ap()

    xt16 = din("xt16", [FIN, NPAD], dt.float16)
    rhsA = din("rhsA", [2, 128, 262], dt.float16)
    waT = din("waT", [2, 128, 128], dt.float16)
    w1T = din("w1T", [128, 64], dt.float16)
    w2T = din("w2T", [64, 32], dt.float16)
    w3T = din("w3T", [32, 3], dt.float16)
    bgat_bc = din("bgat_bc", [128, 256], dt.float16)
    ba_bc = din("ba_bc", [128, 128])
    lnaw_bc = din("lnaw_bc", [128, 128])
    lnab_bc = din("lnab_bc", [128, 128])
    b1_bc = din("b1_bc", [128, 64])
    ln1w_bc = din("ln1w_bc", [128, 64])
    ln1b_bc = din("ln1b_bc", [128, 64])
    b2_bc = din("b2_bc", [128, 32])
    ln2w_bc = din("ln2w_bc", [128, 32])
    ln2b_bc = din("ln2b_bc", [128, 32])
    b3_bc = din("b3_bc", [128, 3])
    hidxI = din("hidxI", [RB, 128, TB], dt.int32)
    ohtab = din("ohtab", [RB, 128, TB * 128], dt.float16)    # [edge_p, t*128+slot]
    ohTtab = din("ohTtab", [RB, 128, TB * 128], dt.float16)  # [slot_p, t*128+edge]
    outD = nc.dram_tensor("outD", [RPAD, NCOL], dt.float16, kind="ExternalOutput").ap()

    class _PhaseDone(Exception):
        pass

    import contextlib
    try:
        _tc_cm = tile.TileContext(nc)
        tc = _tc_cm.__enter__()
        est = contextlib.ExitStack()
        with est:
            top = est.enter_context(tc.tile_pool(name="top", bufs=1))
            dram = est.enter_context(tc.tile_pool(name="dram", bufs=1, space="DRAM"))

            htable = dram.tile([NPAD, HROW], dt.float16, tag="htable")
            HALF = RPAD // 2
            cc_inA = dram.tile([4, HALF], dt.float32, tag="cc_inA")
            cc_outA = dram.tile([CORES, 4, HALF], dt.float32, tag="cc_outA")
            cc_inB = dram.tile([4, HALF], dt.float32, tag="cc_inB")
            cc_outB = dram.tile([CORES, 4, HALF], dt.float32, tag="cc_outB")

            ident = top.tile([128, 128], dt.float32, tag="ident")
            make_identity(nc, ident[:])
            ident16 = top.tile([128, 128], dt.float16, tag="ident16")
            nc.vector.tensor_copy(out=ident16[:], in_=ident[:])
            eps_col = top.tile([128, 1], dt.float32, tag="eps_col")
            nc.vector.memset(eps_col[:], EPS)
            eps4_col = top.tile([128, 1], dt.float32, tag="eps4_col")
            nc.vector.memset(eps4_col[:], 1e-4)

            # ---- load replicated weights / biases into SBUF ----
            def ldw(name, ap, shape, d=dt.float32):
                t = top.tile(shape, d, tag=name)
                nc.sync.dma_start(out=t[:], in_=ap)
                return t

            # ---- phase A: htable rows [h0 | 1 | h1 | 1 | a_s(2) a_d(2)] fp16 ----
            with tc.tile_pool(name="pa", bufs=1) as pa, \
                 tc.tile_pool(name="pa_ht", bufs=4) as pa_ht, \
                 tc.tile_pool(name="pa_h", bufs=4, space="PSUM") as pa_h:
                xta = pa.tile([128, NPAD], dt.float16, tag="xta")
                xtb = pa.tile([128, NPAD], dt.float16, tag="xtb")
                NH = 1280
                rhsA0 = pa.tile([128, 262], dt.float16, tag="rhsA0")
                rhsA1 = pa.tile([128, 262], dt.float16, tag="rhsA1")
                nc.scalar.dma_start(out=rhsA0[:], in_=rhsA[0])
                nc.scalar.dma_start(out=rhsA1[:], in_=rhsA[1])
                nc.sync.dma_start(out=xta[:, 0:NH], in_=xt16[0:128, 0:NH])
                nc.sync.dma_start(out=xtb[:, 0:NH], in_=xt16[128:256, 0:NH])
                nc.sync.dma_start(out=xta[:, NH:NPAD], in_=xt16[0:128, NH:NPAD])
                nc.sync.dma_start(out=xtb[:, NH:NPAD], in_=xt16[128:256, NH:NPAD])
                for i in range(NB):
                    r0 = i * 128
                    hp = pa_h.tile([128, 262], dt.float32, tag="hp", space="PSUM")
                    nc.tensor.matmul(hp[:], xta[:, r0:r0 + 128], rhsA0[:],
                                     start=True, stop=False)
                    nc.tensor.matmul(hp[:], xtb[:, r0:r0 + 128], rhsA1[:],
                                     start=False, stop=True)
                    ht = pa_ht.tile([128, 262], dt.float16, tag="ht")
                    if i % 2 == 0:
                        nc.scalar.copy(out=ht[:], in_=hp[:])
                    else:
                        nc.vector.tensor_copy(out=ht[:], in_=hp[:])
                    nc.gpsimd.memset(ht[:, 128:129], 1.0)
                    nc.gpsimd.memset(ht[:, 257:258], 1.0)
                    nc.sync.dma_start(out=htable[r0:r0 + 128, 0:262], in_=ht[:])

            waT0 = ldw("waT0", waT[0], [128, 128], dt.float16)
            waT1 = ldw("waT1", waT[1], [128, 128], dt.float16)
            w1T_sb = ldw("w1T_sb", w1T[:], [128, 64], dt.float16)
            w2T_sb = ldw("w2T_sb", w2T[:], [64, 32], dt.float16)
            w3T_sb = ldw("w3T_sb", w3T[:], [32, 3], dt.float16)
            bgat_sb = ldw("bgat_sb", bgat_bc[:], [128, 256], dt.float16)
            ba_sb = ldw("ba_sb", ba_bc[:], [128, 128])
            lnaw_sb = ldw("lnaw_sb", lnaw_bc[:], [128, 128])
            lnab_sb = ldw("lnab_sb", lnab_bc[:], [128, 128])
            b1_sb = ldw("b1_sb", b1_bc[:], [128, 64])
            ln1w_sb = ldw("ln1w_sb", ln1w_bc[:], [128, 64])
            ln1b_sb = ldw("ln1b_sb", ln1b_bc[:], [128, 64])
            b2_sb = ldw("b2_sb", b2_bc[:], [128, 32])
            ln2w_sb = ldw("ln2w_sb", ln2w_bc[:], [128, 32])
            ln2b_sb = ldw("ln2b_sb", ln2b_bc[:], [128, 32])
            b3_sb = ldw("b3_sb", b3_bc[:], [128, 3])

            if phases == "A":
                dbg = top.tile([128, 262], dt.float16, tag="dbgA")
                for i in range(RB):
                    nc.sync.dma_start(out=dbg[:], in_=htable[i * 128:(i + 1) * 128, 0:262])
                    nc.sync.dma_start(out=outD[i * 128:(i + 1) * 128, 0:262], in_=dbg[:])
                raise _PhaseDone()

            # ---- phase B: GAT aggregation per dst block ----
            xg_pool = est.enter_context(tc.tile_pool(name="xg", bufs=1))
            xgs = []
            with tc.tile_pool(name="pb_idx", bufs=2) as pb_idx, \
                 tc.tile_pool(name="pb_oht", bufs=3) as pb_oht, \
                 tc.tile_pool(name="pb_g", bufs=6) as pb_g, \
                 tc.tile_pool(name="pb_ad", bufs=2) as pb_ad, \
                 tc.tile_pool(name="pb_ex", bufs=4) as pb_ex, \
                 tc.tile_pool(name="pb_rhs", bufs=6) as pb_rhs, \
                 tc.tile_pool(name="pb_ep", bufs=2) as pb_ep, \
                 tc.tile_pool(name="pb_ps", bufs=2, space="PSUM") as pb_ps, \
                 tc.tile_pool(name="pb_adp", bufs=4, space="PSUM") as pb_adp:
                for _slot in range(6):
                    gz = pb_g.tile([128, GC, HROW], dt.float16, tag="g")
                    nc.vector.memset(gz[:], 0.0)
                for b in range(RB):
                    hix = pb_idx.tile([128, TB], dt.int32, tag="hix")
                    nc.sync.dma_start(out=hix[:], in_=hidxI[b])
                    oh_sb = pb_oht.tile([128, TB * 128], dt.float16, tag="oh_sb")
                    nc.sync.dma_start(out=oh_sb[:], in_=ohtab[b])
                    ohT_sb = pb_oht.tile([128, TB * 128], dt.float16, tag="ohT_sb")
                    nc.sync.dma_start(out=ohT_sb[:], in_=ohTtab[b])

                    adblk = pb_ad.tile([128, 2], dt.float16, tag="adblk")
                    ps = pb_ps.tile([128, 258], dt.float32, tag="ps", space="PSUM")

                    for c in range(NCH):
                        g = pb_g.tile([128, GC, HROW], dt.float16, tag="g")
                        for tl in range(GC):
                            nc.gpsimd.indirect_dma_start(
                                out=g[:, tl, :], out_offset=None,
                                in_=htable[:],
                                in_offset=bass.IndirectOffsetOnAxis(
                                    ap=hix[:, c * GC + tl:c * GC + tl + 1], axis=0))
                        if c == 0:
                            # tile 0 rows are this block's own dst rows (self loops)
                            nc.vector.tensor_copy(out=adblk[:], in_=g[:, 0, 260:262])
                        for hs in range(GC // SUB):
                            t0 = c * GC + hs * SUB
                            adps = pb_adp.tile([128, SUB, 2], dt.float32, tag="adps",
                                               space="PSUM")
                            for k in range(SUB):
                                t = t0 + k
                                nc.tensor.matmul(adps[:, k, :],
                                                 ohT_sb[:, t * 128:(t + 1) * 128],
                                                 adblk[:], start=True, stop=True)
                            # e = a_s[src] + a_d[dst]; exv = max(exp(e), exp(0.2e))
                            exr = pb_ex.tile([128, SUB, 2], dt.float32, tag="exr")
                            nc.vector.tensor_tensor(
                                out=exr[:], in0=adps[:],
                                in1=g[:, hs * SUB:(hs + 1) * SUB, 258:260], op=OP.add)
                            exn = pb_ex.tile([128, SUB, 2], dt.float32, tag="exn")
                            nc.scalar.activation(out=exn[:], in_=exr[:], func=AF.Exp,
                                                 scale=0.2)
                            exp_ = pb_ex.tile([128, SUB, 2], dt.float32, tag="exp_")
                            nc.scalar.activation(out=exp_[:], in_=exr[:], func=AF.Exp)
                            exv = pb_ex.tile([128, SUB, 2], dt.float32, tag="exv")
                            nc.vector.tensor_tensor(out=exv[:], in0=exp_[:], in1=exn[:],
                                                    op=OP.max)
                            for k in range(SUB):
                                t = t0 + k
                                tl = hs * SUB + k
                                rhs = pb_rhs.tile([128, 258], dt.float16, tag="rhs")
                                if t % 2 == 0:
                                    nc.scalar.activation(out=rhs[:, 0:129],
                                                         in_=g[:, tl, 0:129],
                                                         func=AF.Copy,
                                                         scale=exv[:, k, 0:1])
                                    nc.vector.tensor_scalar(
                                        out=rhs[:, 129:258], in0=g[:, tl, 129:258],
                                        scalar1=exv[:, k, 1:2], scalar2=None, op0=OP.mult)
                                else:
                                    nc.vector.tensor_scalar(
                                        out=rhs[:, 0:129], in0=g[:, tl, 0:129],
                                        scalar1=exv[:, k, 0:1], scalar2=None, op0=OP.mult)
                                    nc.scalar.activation(out=rhs[:, 129:258],
                                                         in_=g[:, tl, 129:258],
                                                         func=AF.Copy,
                                                         scale=exv[:, k, 1:2])
                                nc.tensor.matmul(ps[:],
                                                 oh_sb[:, t * 128:(t + 1) * 128],
                                                 rhs[:],
                                                 start=(t == 0), stop=(t == TB - 1))

                    # epilogue: normalize, +b_gat, lrelu(0.01) -> fp16 xg
                    rec0 = pb_ep.tile([128, 1], dt.float32, tag="rec0")
                    nc.vector.reciprocal(rec0[:], ps[:, 128:129])
                    rec1 = pb_ep.tile([128, 1], dt.float32, tag="rec1")
                    nc.vector.reciprocal(rec1[:], ps[:, 257:258])
                    xg = xg_pool.tile([128, 256], dt.float16, tag=f"xg{b}")
                    nc.scalar.activation(out=xg[:, 0:128], in_=ps[:, 0:128],
                                         func=AF.Copy, scale=rec0[:])
                    nc.scalar.activation(out=xg[:, 128:256], in_=ps[:, 129:257],
                                         func=AF.Copy, scale=rec1[:])
                    nc.vector.tensor_tensor(out=xg[:], in0=xg[:], in1=bgat_sb[:], op=OP.add)
                    ng = pb_ep.tile([128, 256], dt.float16, tag="ng")
                    nc.vector.tensor_scalar(out=ng[:], in0=xg[:], scalar1=0.0,
                                            scalar2=0.01, op0=OP.min, op1=OP.mult)
                    nc.vector.scalar_tensor_tensor(out=xg[:], in0=xg[:], scalar=0.0,
                                                   in1=ng[:], op0=OP.max, op1=OP.add)
                    xgs.append(xg)

            if phases == "AB":
                for b in range(RB):
                    nc.sync.dma_start(out=outD[b * 128:(b + 1) * 128, 0:256], in_=xgs[b][:])
                raise _PhaseDone()

            # ---- phase C: dense + LN on own shard (stage-parallel across blocks) ----
            cc_sb = top.tile([4, RPAD], dt.float32, tag="cc_sb")
            with tc.tile_pool(name="pc", bufs=12) as pc, \
                 tc.tile_pool(name="pc_ps", bufs=2, space="PSUM") as pc_ps, \
                 tc.tile_pool(name="pc_mm", bufs=4, space="PSUM") as pc_mm:

                def transpose16(xin, pdim, fdim):
                    # xin fp16 [pdim, fdim] -> sbuf fp16 [fdim, pdim]
                    p = pc_ps.tile([128, 128], dt.float16, tag="tpp16", space="PSUM")
                    nc.tensor.transpose(out=p[:fdim, 0:pdim], in_=xin,
                                        identity=ident16[:pdim, :pdim])
                    s = pc.tile([fdim, pdim], dt.float16, tag=f"tt{fdim}_{pdim}")
                    nc.scalar.copy(out=s[:], in_=p[:fdim, 0:pdim])
                    return s

                def c_chain(b):
                    x0 = xgs[b]
                    xt0 = transpose16(x0[:, 0:128], 128, 128)
                    yield
                    xt1 = transpose16(x0[:, 128:256], 128, 128)
                    yield
                    pA = pc_mm.tile([128, 128], dt.float32, tag="mm", space="PSUM")
                    nc.tensor.matmul(pA[:], xt0[:], waT0[:], start=True, stop=False)
                    nc.tensor.matmul(pA[:], xt1[:], waT1[:], start=False, stop=True)
                    yield
                    x1 = yield from ln_lrelu(b, pA[:], 128, ba_sb, lnaw_sb, lnab_sb)
                    x1t = transpose16(x1[:], 128, 128)
                    yield
                    p1 = pc_mm.tile([128, 64], dt.float32, tag="mm", space="PSUM")
                    nc.tensor.matmul(p1[:], x1t[:], w1T_sb[:], start=True, stop=True)
                    yield
                    x2 = yield from ln_lrelu(b, p1[:], 64, b1_sb, ln1w_sb, ln1b_sb)
                    x2t = transpose16(x2[:], 128, 64)
                    yield
                    p2 = pc_mm.tile([128, 32], dt.float32, tag="mm", space="PSUM")
                    nc.tensor.matmul(p2[:], x2t[:], w2T_sb[:], start=True, stop=True)
                    yield
                    x3 = yield from ln_lrelu(b, p2[:], 32, b2_sb, ln2w_sb, ln2b_sb)
                    x3t = transpose16(x3[:], 128, 32)
                    yield
                    p3 = pc_mm.tile([128, 3], dt.float32, tag="mm", space="PSUM")
                    nc.tensor.matmul(p3[:], x3t[:], w3T_sb[:], start=True, stop=True)
                    yield
                    y3 = pc.tile([128, 4], dt.float32, tag="y3")
                    nc.vector.tensor_tensor(out=y3[:, 0:3], in0=p3[:], in1=b3_sb[:], op=OP.add)
                    scr3 = pc.tile([128, 3], dt.float32, tag="scr3")
                    nc.scalar.activation(out=scr3[:], in_=y3[:, 0:3], func=AF.Square,
                                         accum_out=y3[:, 3:4])
                    yield
                    h3p = pc_ps.tile([128, 128], dt.float32, tag="tpp", space="PSUM")
                    nc.tensor.transpose(out=h3p[:4, 0:128], in_=y3[:], identity=ident[:])
                    nc.scalar.copy(out=cc_sb[:, b * 128:(b + 1) * 128], in_=h3p[:4, 0:128])

                def ln_lrelu(b, xin, fdim, bias_bc, w_bc, b_bc):
                    # y = xin + bias; u = LN(y)*w + b; return lrelu001(u) fp16
                    y = pc.tile([128, fdim], dt.float32, tag=f"y{fdim}")
                    nc.vector.tensor_tensor(out=y[:], in0=xin, in1=bias_bc[:], op=OP.add)
                    yield
                    scr = pc.tile([128, fdim], dt.float32, tag=f"scr{fdim}")
                    msum = pc.tile([128, 1], dt.float32, tag="msum")
                    nc.scalar.activation(out=scr[:], in_=y[:], func=AF.Copy,
                                         accum_out=msum[:])
                    sqs = pc.tile([128, 1], dt.float32, tag="sqs")
                    nc.scalar.activation(out=scr[:], in_=y[:], func=AF.Square,
                                         accum_out=sqs[:])
                    yield
                    mean = pc.tile([128, 1], dt.float32, tag="mean")
                    nc.vector.tensor_scalar(out=mean[:], in0=msum[:], scalar1=1.0 / fdim,
                                            scalar2=None, op0=OP.mult)
                    var = pc.tile([128, 1], dt.float32, tag="var")
                    nc.vector.tensor_scalar(out=var[:], in0=sqs[:], scalar1=1.0 / fdim,
                                            scalar2=None, op0=OP.mult)
                    m2 = pc.tile([128, 1], dt.float32, tag="m2")
                    nc.vector.tensor_scalar(out=m2[:], in0=mean[:], scalar1=mean[:, 0:1],
                                            scalar2=None, op0=OP.mult)
                    nc.vector.tensor_tensor(out=var[:], in0=var[:], in1=m2[:], op=OP.subtract)
                    sd = pc.tile([128, 1], dt.float32, tag="sd")
                    nc.scalar.activation(out=sd[:], in_=var[:], func=AF.Sqrt,
                                         bias=eps_col[:, 0:1])
                    rstd = pc.tile([128, 1], dt.float32, tag="rstd")
                    nc.vector.reciprocal(rstd[:], sd[:])
                    yield
                    u = pc.tile([128, fdim], dt.float32, tag=f"u{fdim}")
                    nc.vector.scalar_tensor_tensor(out=u[:], in0=y[:], scalar=mean[:, 0:1],
                                                   in1=w_bc[:], op0=OP.subtract, op1=OP.mult)
                    nc.vector.scalar_tensor_tensor(out=u[:], in0=u[:], scalar=rstd[:, 0:1],
                                                   in1=b_bc[:], op0=OP.mult, op1=OP.add)
                    yield
                    ngt = pc.tile([128, fdim], dt.float32, tag=f"ng{fdim}")
                    nc.vector.tensor_scalar(out=ngt[:], in0=u[:], scalar1=0.0,
                                            scalar2=0.01, op0=OP.min, op1=OP.mult)
                    u16 = pc.tile([128, fdim], dt.float16, tag=f"u16_{fdim}")
                    nc.vector.scalar_tensor_tensor(out=u16[:], in0=u[:], scalar=0.0,
                                                   in1=ngt[:], op0=OP.max, op1=OP.add)
                    yield
                    return u16

                gens = [c_chain(b) for b in range(RB)]

                def run_gens(idxs):
                    done = {b: False for b in idxs}
                    while not all(done.values()):
                        for b in idxs:
                            if not done[b]:
                                try:
                                    next(gens[b])
                                except StopIteration:
                                    done[b] = True

                run_gens(range(RB // 2))
                nc.sync.dma_start(out=cc_inA[:], in_=cc_sb[:, 0:HALF])
                nc.gpsimd.collective_compute(
                    "AllGather", OP.bypass, replica_groups=[list(range(CORES))],
                    ins=[cc_inA[:].opt()], outs=[cc_outA[:].opt()])
                run_gens(range(RB // 2, RB))
                nc.sync.dma_start(out=cc_inB[:], in_=cc_sb[:, HALF:RPAD])
                nc.gpsimd.collective_compute(
                    "AllGather", OP.bypass, replica_groups=[list(range(CORES))],
                    ins=[cc_inB[:].opt()], outs=[cc_outB[:].opt()])

            if phases == "ABC":
                dbg16 = top.tile([4, RPAD], dt.float16, tag="dbgc")
                nc.vector.tensor_copy(out=dbg16[:], in_=cc_sb[:])
                nc.sync.dma_start(out=outD[0:4, 0:RPAD], in_=dbg16[:])
                raise _PhaseDone()

            # ---- phase D: build split-fp16 cdist operands ----
            # lhsT16 rows: [-2a(3) | -2a(3) | -2b(3) | sqhi | sqlo | 1 | 1]
            # (compute in partition-0 tiles, assemble via sbuf-to-sbuf DMA)
            lhsT16 = top.tile([13, RPAD], dt.float16, tag="lhsT16")
            pd = est.enter_context(tc.tile_pool(name="pd", bufs=1))
            a_own = pd.tile([4, RPAD], dt.float16, tag="a_own")
            nc.vector.tensor_copy(out=a_own[:], in_=cc_sb[:])
            b_own = pd.tile([4, RPAD], dt.float16, tag="b_own")
            nc.vector.tensor_tensor(out=b_own[:], in0=cc_sb[:], in1=a_own[:],
                                    op=OP.subtract)
            na4 = pd.tile([4, RPAD], dt.float16, tag="na4")
            nc.scalar.activation(out=na4[:], in_=a_own[:], func=AF.Copy, scale=-2.0)
            nb4 = pd.tile([4, RPAD], dt.float16, tag="nb4")
            nc.scalar.activation(out=nb4[:], in_=b_own[:], func=AF.Copy, scale=-2.0)
            ones_r = pd.tile([2, RPAD], dt.float16, tag="ones_r")
            nc.vector.memset(ones_r[:], 1.0)
            nc.sync.dma_start(out=lhsT16[0:3, :], in_=na4[0:3, :])
            nc.sync.dma_start(out=lhsT16[3:6, :], in_=na4[0:3, :])
            nc.sync.dma_start(out=lhsT16[6:9, :], in_=nb4[0:3, :])
            nc.sync.dma_start(out=lhsT16[9:10, :], in_=a_own[3:4, :])
            nc.sync.dma_start(out=lhsT16[10:11, :], in_=b_own[3:4, :])
            nc.sync.dma_start(out=lhsT16[11:13, :], in_=ones_r[:])

            # rhs16 rows: [a(3) | b(3) | a(3) | 1 | 1 | sqhi | sqlo]
            rhs_f = pd.tile([4, NCOL], dt.float32, tag="rhs_f")
            nc.vector.memset(rhs_f[:, N:NCOL], 0.0)
            a16 = pd.tile([4, NCOL], dt.float16, tag="a16")
            b16 = pd.tile([4, NCOL], dt.float16, tag="b16")
            vA = lambda t, r0, r1: t[r0:r1, 0:10000].rearrange(
                "p (s c) -> p s c", s=8)[:, :, 0:HALF]
            vB = lambda t, r0, r1: t[r0:r1, 0:10000].rearrange(
                "p (s c) -> p s c", s=8)[:, :, HALF:SHARD]
            # half A (ready while collective B still running)
            for s in range(CORES):
                c0 = s * SHARD
                nc.sync.dma_start(out=rhs_f[0:4, c0:c0 + HALF],
                                  in_=cc_outA[:][s])
            nc.vector.tensor_copy(out=vA(a16, 0, 4), in_=vA(rhs_f, 0, 4))
            nc.vector.tensor_tensor(out=vA(b16, 0, 4), in0=vA(rhs_f, 0, 4),
                                    in1=vA(a16, 0, 4), op=OP.subtract)
            # half B
            for s in range(CORES):
                c0 = s * SHARD
                nc.sync.dma_start(out=rhs_f[0:4, c0 + HALF:c0 + SHARD],
                                  in_=cc_outB[:][s, 0:4, 0:SHARD - HALF])
            nc.vector.tensor_copy(out=vB(a16, 0, 4), in_=vB(rhs_f, 0, 4))
            nc.vector.tensor_tensor(out=vB(b16, 0, 4), in0=vB(rhs_f, 0, 4),
                                    in1=vB(a16, 0, 4), op=OP.subtract)
            # pad cols 10000:10240
            nc.scalar.copy(out=a16[:, N:NCOL], in_=rhs_f[:, N:NCOL])
            nc.scalar.copy(out=b16[:, N:NCOL], in_=rhs_f[:, N:NCOL])
            ones_n = pd.tile([2, NCOL], dt.float16, tag="ones_n")
            nc.vector.memset(ones_n[:], 1.0)
            rhs16 = top.tile([13, NCOL], dt.float16, tag="rhs16")
            nc.sync.dma_start(out=rhs16[0:3, :], in_=a16[0:3, :])
            nc.sync.dma_start(out=rhs16[3:6, :], in_=b16[0:3, :])
            nc.sync.dma_start(out=rhs16[6:9, :], in_=a16[0:3, :])
            nc.sync.dma_start(out=rhs16[9:11, :], in_=ones_n[:])
            nc.sync.dma_start(out=rhs16[11:12, :], in_=a16[3:4, :])
            nc.sync.dma_start(out=rhs16[12:13, :], in_=b16[3:4, :])

            if phases == "ABCD":
                nc.sync.dma_start(out=outD[0:13, 0:NCOL], in_=rhs16[:])
                raise _PhaseDone()

            # ---- phase E: cdist row-block x col-chunk (split-fp16 matmul, fp16 out) ----
            MRG = 4   # psum chunks merged into one output tile/DMA
            with tc.tile_pool(name="pe_d", bufs=3) as pe_d, \
                 tc.tile_pool(name="pe_d2", bufs=3) as pe_d2, \
                 tc.tile_pool(name="pe_ps", bufs=6, space="PSUM") as pe_ps:
                for rb in range(RB):
                    for mg in range(NCOL // (CCH * MRG)):
                        d2t = pe_d2.tile([128, CCH * MRG], dt.float16, tag="d2t")
                        for k in range(MRG):
                            ch = mg * MRG + k
                            dp = pe_ps.tile([128, CCH], dt.float32, tag="dp", space="PSUM")
                            nc.tensor.matmul(
                                dp[:], lhsT16[:, rb * 128:(rb + 1) * 128],
                                rhs16[:, ch * CCH:(ch + 1) * CCH],
                                start=True, stop=True)
                            nc.scalar.activation(out=d2t[:, k * CCH:(k + 1) * CCH],
                                                 in_=dp[:], func=AF.Sqrt,
                                                 bias=eps4_col[:, 0:1])
                        nc.sync.dma_start(
                            out=outD[rb * 128:(rb + 1) * 128,
                                     mg * CCH * MRG:(mg + 1) * CCH * MRG],
                            in_=d2t[:])

    except _PhaseDone:
        pass
    _tc_cm.__exit__(None, None, None)
    nc.compile()
    return nc


def _prep_host(x, edge_index):
    xp = np.zeros((NPAD, FIN), np.float32)
    xp[:N] = np.asarray(x, np.float32)
    xp16 = np.ascontiguousarray(xp.T.astype(np.float16))  # [256, NPAD]

    ei = np.asarray(edge_index)
    src = ei[0].astype(np.int64)
    dst = ei[1].astype(np.int64)

    core = dst // SHARD
    per_core = []
    max_tiles = 0
    for c in range(CORES):
        sel = core == c
        s_c = src[sel]
        d_c = dst[sel]
        loc = d_c - c * SHARD
        blk = loc // 128
        dl = loc - blk * 128
        blocks = []
        for b in range(RB):
            m = blk == b
            blocks.append((s_c[m], dl[m]))
            # tile 0 holds the block's self-loop edges; rest start at tile 1
            max_tiles = max(max_tiles, 1 + (len(blocks[-1][0]) + 127) // 128)
        per_core.append(blocks)

    TB = GC * ((max_tiles + GC - 1) // GC)
    S16 = 8 * TB
    NE = TB * 128

    hidx = np.zeros((CORES, RB, 128, TB), np.int32)
    ohtab = np.zeros((CORES, RB, 128, NE), np.float16)
    ohTtab = np.zeros((CORES, RB, 128, NE), np.float16)
    for c in range(CORES):
        for b in range(RB):
            # tile 0: self loops (edge at partition p has src=dst=block row p)
            rows = c * SHARD + b * 128 + np.arange(128)
            real = rows < N
            crows = np.minimum(rows, N - 1)
            jr = np.arange(128)
            hidx[c, b, jr, 0] = crows.astype(np.int32)
            pr = jr[real]
            ohtab[c, b, pr, pr] = 1.0
            ohTtab[c, b, pr, pr] = 1.0
            # remaining edges from tile 1 on
            s_b, dl_b = per_core[c][b]
            n = len(s_b)
            js = 128 + np.arange(n)
            p = js % 128          # edge partition
            t = js // 128         # edge tile (>= 1)
            hidx[c, b, p, t] = s_b.astype(np.int32)
            sl = dl_b.astype(np.int64)
            ohtab[c, b, p, t * 128 + sl] = 1.0
            ohTtab[c, b, sl, t * 128 + p] = 1.0
    return xp16, hidx, ohtab, ohTtab, TB


def build_in_maps(inputs):
    xp16, hidx, ohtab, ohTtab, TB = _prep_host(inputs["x"], inputs["edge_index"])

    def bc(vec, n, f16=False):
        v = np.asarray(vec, np.float32).reshape(1, n)
        out = np.ascontiguousarray(np.broadcast_to(v, (128, n)).copy())
        return out.astype(np.float16) if f16 else out

    # rhsA: [256 (xfeat, 2 chunks of 128), 262] fp16
    # cols: 0:128 WgT head0 | 128 zero | 129:257 WgT head1 | 257 zero | 258:262 wtil
    Wg = np.asarray(inputs["W_gat"], np.float32)       # [256, 256] rows = H*F out
    att_src = np.asarray(inputs["att_src"], np.float32)  # [2, 128]
    att_dst = np.asarray(inputs["att_dst"], np.float32)
    rhsA = np.zeros((256, 262), np.float32)
    rhsA[:, 0:128] = Wg[0:128, :].T
    rhsA[:, 129:257] = Wg[128:256, :].T
    rhsA[:, 258] = Wg[0:128, :].T @ att_src[0]
    rhsA[:, 259] = Wg[128:256, :].T @ att_src[1]
    rhsA[:, 260] = Wg[0:128, :].T @ att_dst[0]
    rhsA[:, 261] = Wg[128:256, :].T @ att_dst[1]
    rhsA16 = rhsA.astype(np.float16).reshape(2, 128, 262)

    Wa = np.asarray(inputs["Wa"], np.float32)  # [128, 256]
    waT16 = np.ascontiguousarray(Wa.T.astype(np.float16)).reshape(2, 128, 128)
    w1T16 = np.ascontiguousarray(np.asarray(inputs["W1"], np.float32).T.astype(np.float16))
    w2T16 = np.ascontiguousarray(np.asarray(inputs["W2"], np.float32).T.astype(np.float16))
    w3T16 = np.ascontiguousarray(np.asarray(inputs["W3"], np.float32).T.astype(np.float16))

    shared = {
        "xt16": xp16,
        "rhsA": np.ascontiguousarray(rhsA16),
        "waT": waT16, "w1T": w1T16, "w2T": w2T16, "w3T": w3T16,
        "bgat_bc": bc(inputs["b_gat"], 256, f16=True),
        "ba_bc": bc(inputs["ba"], 128),
        "lnaw_bc": bc(inputs["lna_w"], 128), "lnab_bc": bc(inputs["lna_b"], 128),
        "b1_bc": bc(inputs["b1"], 64),
        "ln1w_bc": bc(inputs["ln1_w"], 64), "ln1b_bc": bc(inputs["ln1_b"], 64),
        "b2_bc": bc(inputs["b2"], 32),
        "ln2w_bc": bc(inputs["ln2_w"], 32), "ln2b_bc": bc(inputs["ln2_b"], 32),
        "b3_bc": bc(inputs["b3"], 3),
    }
    in_maps = [
        {**shared, "hidxI": np.ascontiguousarray(hidx[c]),
         "ohtab": np.ascontiguousarray(ohtab[c]),
         "ohTtab": np.ascontiguousarray(ohTtab[c])}
        for c in range(CORES)
    ]
    return in_maps, TB


def kernel(**inputs):
    in_maps, TB = build_in_maps(inputs)

    import os
    phases = os.environ.get("K_PHASES", "ABCDE")
    key = (TB, phases)
    if key not in _BUILD_CACHE:
        _BUILD_CACHE[key] = _build(TB, phases)
    nc = _BUILD_CACHE[key]
    res = run_bass_kernel_spmd(nc, in_maps, core_ids=list(range(CORES)))
    global _LAST_RESULTS
    _LAST_RESULTS = res.results
    out = np.empty((N, N), np.float32)
    for c in range(CORES):
        out[c * SHARD:(c + 1) * SHARD, :] = \
            res.results[c]["outD"][:SHARD, :N].astype(np.float32)
    return out


# revision 23
# speedup vs baseline: 1.1607x; 1.1607x over previous
"""GAT (2-head) + 3x dense/LayerNorm + pairwise-distance kernel for 8 TRN2 NeuronCores.

Strategy: dst-sharded edge processing (one-hot matmul aggregation), replicated
small dense weights, row-block-sharded NxN cdist output.

v3: fp16 htable/gather rows, host-precomputed one-hot tables (no on-device
is_eq), 2-queue GC=12 gathers, whole-x preload, split-fp16 cdist matmuls,
fp16 output.
"""
import sys

import numpy as np

# Environment bootstrap (harness may run from a bare directory).
for _p in ("/root/.axon_site", "/root/.axon_site/_ro/trn_rl_repo",
           "/root/.axon_site/_ro/pypackages", "/opt/trn_rl_repo"):
    if _p not in sys.path:
        sys.path.append(_p)

import concourse.bass as bass
import concourse.bacc as bacc
import concourse.mybir as mybir
import concourse.tile as tile
from concourse.masks import make_identity
from concourse.bass_utils import run_bass_kernel_spmd

dt = mybir.dt
OP = mybir.AluOpType
AF = mybir.ActivationFunctionType

N = 10000
NPAD = 10112          # 79 * 128
NB = 79               # node blocks (phase A)
FIN = 256
F = 128               # per-head GAT dim
H = 2
HROW = 384            # htable row fp16 elems (768B, multiple of 256B)
CORES = 8
SHARD = 1250          # dst nodes per core
RB = 10               # dst blocks per core
RPAD = 1280
CCH = 512             # cdist column chunk
NCOL = 10240          # padded output columns
EPS = 1e-5

_BUILD_CACHE = {}
_LAST_RESULTS = None


GC = 6   # tile groups per dma_gather call (768 idxs = 48 descs/engine)
SUB = 3  # tiles per a_d/exp subchunk


def _build(TB, phases="ABCDE"):
    """Build the 8-core SPMD program. TB = gather tile groups per dst block (mult of GC)."""
    assert TB % GC == 0
    NCH = TB // GC
    S16 = 8 * TB          # idx columns ([128, S16] wrapped int16)

    nc = bacc.Bacc("TRN2", target_bir_lowering=False, debug=False,
                   num_devices=CORES, num_swdge_queues=2)

    def din(name, shape, d=dt.float32):
        return nc.dram_tensor(name, shape, d, kind="ExternalInput").ap()

    xt16 = din("xt16", [FIN, NPAD], dt.float16)
    rhsA = din("rhsA", [2, 128, 262], dt.float16)
    waT = din("waT", [2, 128, 128], dt.float16)
    w1T = din("w1T", [128, 64], dt.float16)
    w2T = din("w2T", [64, 32], dt.float16)
    w3T = din("w3T", [32, 3], dt.float16)
    bgat_bc = din("bgat_bc", [128, 256], dt.float16)
    ba_bc = din("ba_bc", [128, 128])
    lnaw_bc = din("lnaw_bc", [128, 128])
    lnab_bc = din("lnab_bc", [128, 128])
    b1_bc = din("b1_bc", [128, 64])
    ln1w_bc = din("ln1w_bc", [128, 64])
    ln1b_bc = din("ln1b_bc", [128, 64])
    b2_bc = din("b2_bc", [128, 32])
    ln2w_bc = din("ln2w_bc", [128, 32])
    ln2b_bc = din("ln2b_bc", [128, 32])
    b3_bc = din("b3_bc", [128, 3])
    hidxI = din("hidxI", [RB, 128, TB], dt.int32)
    ohtab = din("ohtab", [RB, 128, TB * 128], dt.float16)    # [edge_p, t*128+slot]
    ohTtab = din("ohTtab", [RB, 128, TB * 128], dt.float16)  # [slot_p, t*128+edge]
    outD = nc.dram_tensor("outD", [RPAD, NCOL], dt.float16, kind="ExternalOutput").ap()

    class _PhaseDone(Exception):
        pass

    import contextlib
    try:
        _tc_cm = tile.TileContext(nc)
        tc = _tc_cm.__enter__()
        est = contextlib.ExitStack()
        with est:
            top = est.enter_context(tc.tile_pool(name="top", bufs=1))
            dram = est.enter_context(tc.tile_pool(name="dram", bufs=1, space="DRAM"))

            htable = dram.tile([NPAD, HROW], dt.float16, tag="htable")
            HALF = RPAD // 2
            cc_inA = dram.tile([4, HALF], dt.float32, tag="cc_inA")
            cc_outA = dram.tile([CORES, 4, HALF], dt.float32, tag="cc_outA")
            cc_inB = dram.tile([4, HALF], dt.float32, tag="cc_inB")
            cc_outB = dram.tile([CORES, 4, HALF], dt.float32, tag="cc_outB")

            ident = top.tile([128, 128], dt.float32, tag="ident")
            make_identity(nc, ident[:])
            ident16 = top.tile([128, 128], dt.float16, tag="ident16")
            nc.vector.tensor_copy(out=ident16[:], in_=ident[:])
            eps_col = top.tile([128, 1], dt.float32, tag="eps_col")
            nc.vector.memset(eps_col[:], EPS)
            eps4_col = top.tile([128, 1], dt.float32, tag="eps4_col")
            nc.vector.memset(eps4_col[:], 1e-4)

            # ---- load replicated weights / biases into SBUF ----
            def ldw(name, ap, shape, d=dt.float32):
                t = top.tile(shape, d, tag=name)
                nc.sync.dma_start(out=t[:], in_=ap)
                return t

            rhsA0 = ldw("rhsA0", rhsA[0], [128, 262], dt.float16)
            rhsA1 = ldw("rhsA1", rhsA[1], [128, 262], dt.float16)
            waT0 = ldw("waT0", waT[0], [128, 128], dt.float16)
            waT1 = ldw("waT1", waT[1], [128, 128], dt.float16)
            w1T_sb = ldw("w1T_sb", w1T[:], [128, 64], dt.float16)
            w2T_sb = ldw("w2T_sb", w2T[:], [64, 32], dt.float16)
            w3T_sb = ldw("w3T_sb", w3T[:], [32, 3], dt.float16)
            bgat_sb = ldw("bgat_sb", bgat_bc[:], [128, 256], dt.float16)
            ba_sb = ldw("ba_sb", ba_bc[:], [128, 128])
            lnaw_sb = ldw("lnaw_sb", lnaw_bc[:], [128, 128])
            lnab_sb = ldw("lnab_sb", lnab_bc[:], [128, 128])
            b1_sb = ldw("b1_sb", b1_bc[:], [128, 64])
            ln1w_sb = ldw("ln1w_sb", ln1w_bc[:], [128, 64])
            ln1b_sb = ldw("ln1b_sb", ln1b_bc[:], [128, 64])
            b2_sb = ldw("b2_sb", b2_bc[:], [128, 32])
            ln2w_sb = ldw("ln2w_sb", ln2w_bc[:], [128, 32])
            ln2b_sb = ldw("ln2b_sb", ln2b_bc[:], [128, 32])
            b3_sb = ldw("b3_sb", b3_bc[:], [128, 3])

            # ---- phase A: htable rows [h0 | 1 | h1 | 1 | a_s(2) a_d(2)] fp16 ----
            with tc.tile_pool(name="pa", bufs=1) as pa, \
                 tc.tile_pool(name="pa_ht", bufs=4) as pa_ht, \
                 tc.tile_pool(name="pa_h", bufs=4, space="PSUM") as pa_h:
                xta = pa.tile([128, NPAD], dt.float16, tag="xta")
                xtb = pa.tile([128, NPAD], dt.float16, tag="xtb")
                NH = 1280
                nc.sync.dma_start(out=xta[:, 0:NH], in_=xt16[0:128, 0:NH])
                nc.sync.dma_start(out=xtb[:, 0:NH], in_=xt16[128:256, 0:NH])
                nc.sync.dma_start(out=xta[:, NH:NPAD], in_=xt16[0:128, NH:NPAD])
                nc.sync.dma_start(out=xtb[:, NH:NPAD], in_=xt16[128:256, NH:NPAD])
                for i in range(NB):
                    r0 = i * 128
                    hp = pa_h.tile([128, 262], dt.float32, tag="hp", space="PSUM")
                    nc.tensor.matmul(hp[:], xta[:, r0:r0 + 128], rhsA0[:],
                                     start=True, stop=False)
                    nc.tensor.matmul(hp[:], xtb[:, r0:r0 + 128], rhsA1[:],
                                     start=False, stop=True)
                    ht = pa_ht.tile([128, 262], dt.float16, tag="ht")
                    if i % 2 == 0:
                        nc.scalar.copy(out=ht[:], in_=hp[:])
                    else:
                        nc.vector.tensor_copy(out=ht[:], in_=hp[:])
                    nc.gpsimd.memset(ht[:, 128:129], 1.0)
                    nc.gpsimd.memset(ht[:, 257:258], 1.0)
                    nc.sync.dma_start(out=htable[r0:r0 + 128, 0:262], in_=ht[:])

            if phases == "A":
                dbg = top.tile([128, 262], dt.float16, tag="dbgA")
                for i in range(RB):
                    nc.sync.dma_start(out=dbg[:], in_=htable[i * 128:(i + 1) * 128, 0:262])
                    nc.sync.dma_start(out=outD[i * 128:(i + 1) * 128, 0:262], in_=dbg[:])
                raise _PhaseDone()

            # ---- phase B: GAT aggregation per dst block ----
            xg_pool = est.enter_context(tc.tile_pool(name="xg", bufs=1))
            xgs = []
            with tc.tile_pool(name="pb_idx", bufs=2) as pb_idx, \
                 tc.tile_pool(name="pb_oht", bufs=3) as pb_oht, \
                 tc.tile_pool(name="pb_g", bufs=6) as pb_g, \
                 tc.tile_pool(name="pb_ad", bufs=2) as pb_ad, \
                 tc.tile_pool(name="pb_ex", bufs=4) as pb_ex, \
                 tc.tile_pool(name="pb_rhs", bufs=6) as pb_rhs, \
                 tc.tile_pool(name="pb_ep", bufs=2) as pb_ep, \
                 tc.tile_pool(name="pb_ps", bufs=2, space="PSUM") as pb_ps, \
                 tc.tile_pool(name="pb_adp", bufs=4, space="PSUM") as pb_adp:
                for _slot in range(6):
                    gz = pb_g.tile([128, GC, HROW], dt.float16, tag="g")
                    nc.vector.memset(gz[:], 0.0)
                for b in range(RB):
                    hix = pb_idx.tile([128, TB], dt.int32, tag="hix")
                    nc.sync.dma_start(out=hix[:], in_=hidxI[b])
                    oh_sb = pb_oht.tile([128, TB * 128], dt.float16, tag="oh_sb")
                    nc.sync.dma_start(out=oh_sb[:], in_=ohtab[b])
                    ohT_sb = pb_oht.tile([128, TB * 128], dt.float16, tag="ohT_sb")
                    nc.sync.dma_start(out=ohT_sb[:], in_=ohTtab[b])

                    adblk = pb_ad.tile([128, 2], dt.float16, tag="adblk")
                    ps = pb_ps.tile([128, 258], dt.float32, tag="ps", space="PSUM")

                    for c in range(NCH):
                        g = pb_g.tile([128, GC, HROW], dt.float16, tag="g")
                        for tl in range(GC):
                            nc.gpsimd.indirect_dma_start(
                                out=g[:, tl, :], out_offset=None,
                                in_=htable[:],
                                in_offset=bass.IndirectOffsetOnAxis(
                                    ap=hix[:, c * GC + tl:c * GC + tl + 1], axis=0))
                        if c == 0:
                            # tile 0 rows are this block's own dst rows (self loops)
                            nc.vector.tensor_copy(out=adblk[:], in_=g[:, 0, 260:262])
                        for hs in range(GC // SUB):
                            t0 = c * GC + hs * SUB
                            adps = pb_adp.tile([128, SUB, 2], dt.float32, tag="adps",
                                               space="PSUM")
                            for k in range(SUB):
                                t = t0 + k
                                nc.tensor.matmul(adps[:, k, :],
                                                 ohT_sb[:, t * 128:(t + 1) * 128],
                                                 adblk[:], start=True, stop=True)
                            # e = a_s[src] + a_d[dst]; exv = max(exp(e), exp(0.2e))
                            exr = pb_ex.tile([128, SUB, 2], dt.float32, tag="exr")
                            nc.vector.tensor_tensor(
                                out=exr[:], in0=adps[:],
                                in1=g[:, hs * SUB:(hs + 1) * SUB, 258:260], op=OP.add)
                            exn = pb_ex.tile([128, SUB, 2], dt.float32, tag="exn")
                            nc.scalar.activation(out=exn[:], in_=exr[:], func=AF.Exp,
                                                 scale=0.2)
                            exp_ = pb_ex.tile([128, SUB, 2], dt.float32, tag="exp_")
                            nc.scalar.activation(out=exp_[:], in_=exr[:], func=AF.Exp)
                            exv = pb_ex.tile([128, SUB, 2], dt.float32, tag="exv")
                            nc.vector.tensor_tensor(out=exv[:], in0=exp_[:], in1=exn[:],
                                                    op=OP.max)
                            for k in range(SUB):
                                t = t0 + k
                                tl = hs * SUB + k
                                rhs = pb_rhs.tile([128, 258], dt.float16, tag="rhs")
                                if t % 2 == 0:
                                    nc.scalar.activation(out=rhs[:, 0:129],
                                                         in_=g[:, tl, 0:129],
                                                         func=AF.Copy,
                                                         scale=exv[:, k, 0:1])
                                    nc.vector.tensor_scalar(
                                        out=rhs[:, 129:258], in0=g[:, tl, 129:258],
                                        scalar1=exv[:, k, 1:2], scalar2=None, op0=OP.mult)
                                else:
                                    nc.vector.tensor_scalar(
                                        out=rhs[:, 0:129], in0=g[:, tl, 0:129],
                                        scalar1=exv[:, k, 0:1], scalar2=None, op0=OP.mult)
                                    nc.scalar.activation(out=rhs[:, 129:258],
                                                         in_=g[:, tl, 129:258],
                                                         func=AF.Copy,
                                                         scale=exv[:, k, 1:2])
                                nc.tensor.matmul(ps[:],
                                                 oh_sb[:, t * 128:(t + 1) * 128],
                                                 rhs[:],
                                                 start=(t == 0), stop=(t == TB - 1))

                    # epilogue: normalize, +b_gat, lrelu(0.01) -> fp16 xg
                    rec0 = pb_ep.tile([128, 1], dt.float32, tag="rec0")
                    nc.vector.reciprocal(rec0[:], ps[:, 128:129])
                    rec1 = pb_ep.tile([128, 1], dt.float32, tag="rec1")
                    nc.vector.reciprocal(rec1[:], ps[:, 257:258])
                    xg = xg_pool.tile([128, 256], dt.float16, tag=f"xg{b}")
                    nc.scalar.activation(out=xg[:, 0:128], in_=ps[:, 0:128],
                                         func=AF.Copy, scale=rec0[:])
                    nc.scalar.activation(out=xg[:, 128:256], in_=ps[:, 129:257],
                                         func=AF.Copy, scale=rec1[:])
                    nc.vector.tensor_tensor(out=xg[:], in0=xg[:], in1=bgat_sb[:], op=OP.add)
                    ng = pb_ep.tile([128, 256], dt.float16, tag="ng")
                    nc.vector.tensor_scalar(out=ng[:], in0=xg[:], scalar1=0.0,
                                            scalar2=0.01, op0=OP.min, op1=OP.mult)
                    nc.vector.scalar_tensor_tensor(out=xg[:], in0=xg[:], scalar=0.0,
                                                   in1=ng[:], op0=OP.max, op1=OP.add)
                    xgs.append(xg)

            if phases == "AB":
                for b in range(RB):
                    nc.sync.dma_start(out=outD[b * 128:(b + 1) * 128, 0:256], in_=xgs[b][:])
                raise _PhaseDone()

            # ---- phase C: dense + LN on own shard (stage-parallel across blocks) ----
            cc_sb = top.tile([4, RPAD], dt.float32, tag="cc_sb")
            with tc.tile_pool(name="pc", bufs=12) as pc, \
                 tc.tile_pool(name="pc_ps", bufs=2, space="PSUM") as pc_ps, \
                 tc.tile_pool(name="pc_mm", bufs=4, space="PSUM") as pc_mm:

                def transpose16(xin, pdim, fdim):
                    # xin fp16 [pdim, fdim] -> sbuf fp16 [fdim, pdim]
                    p = pc_ps.tile([128, 128], dt.float16, tag="tpp16", space="PSUM")
                    nc.tensor.transpose(out=p[:fdim, 0:pdim], in_=xin,
                                        identity=ident16[:pdim, :pdim])
                    s = pc.tile([fdim, pdim], dt.float16, tag=f"tt{fdim}_{pdim}")
                    nc.scalar.copy(out=s[:], in_=p[:fdim, 0:pdim])
                    return s

                def c_chain(b):
                    x0 = xgs[b]
                    xt0 = transpose16(x0[:, 0:128], 128, 128)
                    yield
                    xt1 = transpose16(x0[:, 128:256], 128, 128)
                    yield
                    pA = pc_mm.tile([128, 128], dt.float32, tag="mm", space="PSUM")
                    nc.tensor.matmul(pA[:], xt0[:], waT0[:], start=True, stop=False)
                    nc.tensor.matmul(pA[:], xt1[:], waT1[:], start=False, stop=True)
                    yield
                    x1 = yield from ln_lrelu(b, pA[:], 128, ba_sb, lnaw_sb, lnab_sb)
                    x1t = transpose16(x1[:], 128, 128)
                    yield
                    p1 = pc_mm.tile([128, 64], dt.float32, tag="mm", space="PSUM")
                    nc.tensor.matmul(p1[:], x1t[:], w1T_sb[:], start=True, stop=True)
                    yield
                    x2 = yield from ln_lrelu(b, p1[:], 64, b1_sb, ln1w_sb, ln1b_sb)
                    x2t = transpose16(x2[:], 128, 64)
                    yield
                    p2 = pc_mm.tile([128, 32], dt.float32, tag="mm", space="PSUM")
                    nc.tensor.matmul(p2[:], x2t[:], w2T_sb[:], start=True, stop=True)
                    yield
                    x3 = yield from ln_lrelu(b, p2[:], 32, b2_sb, ln2w_sb, ln2b_sb)
                    x3t = transpose16(x3[:], 128, 32)
                    yield
                    p3 = pc_mm.tile([128, 3], dt.float32, tag="mm", space="PSUM")
                    nc.tensor.matmul(p3[:], x3t[:], w3T_sb[:], start=True, stop=True)
                    yield
                    y3 = pc.tile([128, 4], dt.float32, tag="y3")
                    nc.vector.tensor_tensor(out=y3[:, 0:3], in0=p3[:], in1=b3_sb[:], op=OP.add)
                    scr3 = pc.tile([128, 3], dt.float32, tag="scr3")
                    nc.scalar.activation(out=scr3[:], in_=y3[:, 0:3], func=AF.Square,
                                         accum_out=y3[:, 3:4])
                    yield
                    h3p = pc_ps.tile([128, 128], dt.float32, tag="tpp", space="PSUM")
                    nc.tensor.transpose(out=h3p[:4, 0:128], in_=y3[:], identity=ident[:])
                    nc.scalar.copy(out=cc_sb[:, b * 128:(b + 1) * 128], in_=h3p[:4, 0:128])

                def ln_lrelu(b, xin, fdim, bias_bc, w_bc, b_bc):
                    # y = xin + bias; u = LN(y)*w + b; return lrelu001(u) fp16
                    y = pc.tile([128, fdim], dt.float32, tag=f"y{fdim}")
                    nc.vector.tensor_tensor(out=y[:], in0=xin, in1=bias_bc[:], op=OP.add)
                    yield
                    scr = pc.tile([128, fdim], dt.float32, tag=f"scr{fdim}")
                    msum = pc.tile([128, 1], dt.float32, tag="msum")
                    nc.scalar.activation(out=scr[:], in_=y[:], func=AF.Copy,
                                         accum_out=msum[:])
                    sqs = pc.tile([128, 1], dt.float32, tag="sqs")
                    nc.scalar.activation(out=scr[:], in_=y[:], func=AF.Square,
                                         accum_out=sqs[:])
                    yield
                    mean = pc.tile([128, 1], dt.float32, tag="mean")
                    nc.vector.tensor_scalar(out=mean[:], in0=msum[:], scalar1=1.0 / fdim,
                                            scalar2=None, op0=OP.mult)
                    var = pc.tile([128, 1], dt.float32, tag="var")
                    nc.vector.tensor_scalar(out=var[:], in0=sqs[:], scalar1=1.0 / fdim,
                                            scalar2=None, op0=OP.mult)
                    m2 = pc.tile([128, 1], dt.float32, tag="m2")
                    nc.vector.tensor_scalar(out=m2[:], in0=mean[:], scalar1=mean[:, 0:1],
                                            scalar2=None, op0=OP.mult)
                    nc.vector.tensor_tensor(out=var[:], in0=var[:], in1=m2[:], op=OP.subtract)
                    sd = pc.tile([128, 1], dt.float32, tag="sd")
                    nc.scalar.activation(out=sd[:], in_=var[:], func=AF.Sqrt,
                                         bias=eps_col[:, 0:1])
                    rstd = pc.tile([128, 1], dt.float32, tag="rstd")
                    nc.vector.reciprocal(rstd[:], sd[:])
                    yield
                    u = pc.tile([128, fdim], dt.float32, tag=f"u{fdim}")
                    nc.vector.scalar_tensor_tensor(out=u[:], in0=y[:], scalar=mean[:, 0:1],
                                                   in1=w_bc[:], op0=OP.subtract, op1=OP.mult)
                    nc.vector.scalar_tensor_tensor(out=u[:], in0=u[:], scalar=rstd[:, 0:1],
                                                   in1=b_bc[:], op0=OP.mult, op1=OP.add)
                    yield
                    ngt = pc.tile([128, fdim], dt.float32, tag=f"ng{fdim}")
                    nc.vector.tensor_scalar(out=ngt[:], in0=u[:], scalar1=0.0,
                                            scalar2=0.01, op0=OP.min, op1=OP.mult)
                    u16 = pc.tile([128, fdim], dt.float16, tag=f"u16_{fdim}")
                    nc.vector.scalar_tensor_tensor(out=u16[:], in0=u[:], scalar=0.0,
                                                   in1=ngt[:], op0=OP.max, op1=OP.add)
                    yield
                    return u16

                gens = [c_chain(b) for b in range(RB)]

                def run_gens(idxs):
                    done = {b: False for b in idxs}
                    while not all(done.values()):
                        for b in idxs:
                            if not done[b]:
                                try:
                                    next(gens[b])
                                except StopIteration:
                                    done[b] = True

                run_gens(range(RB // 2))
                nc.sync.dma_start(out=cc_inA[:], in_=cc_sb[:, 0:HALF])
                nc.gpsimd.collective_compute(
                    "AllGather", OP.bypass, replica_groups=[list(range(CORES))],
                    ins=[cc_inA[:].opt()], outs=[cc_outA[:].opt()])
                run_gens(range(RB // 2, RB))
                nc.sync.dma_start(out=cc_inB[:], in_=cc_sb[:, HALF:RPAD])
                nc.gpsimd.collective_compute(
                    "AllGather", OP.bypass, replica_groups=[list(range(CORES))],
                    ins=[cc_inB[:].opt()], outs=[cc_outB[:].opt()])

            if phases == "ABC":
                dbg16 = top.tile([4, RPAD], dt.float16, tag="dbgc")
                nc.vector.tensor_copy(out=dbg16[:], in_=cc_sb[:])
                nc.sync.dma_start(out=outD[0:4, 0:RPAD], in_=dbg16[:])
                raise _PhaseDone()

            # ---- phase D: build split-fp16 cdist operands ----
            # lhsT16 rows: [-2a(3) | -2a(3) | -2b(3) | sqhi | sqlo | 1 | 1]
            # (compute in partition-0 tiles, assemble via sbuf-to-sbuf DMA)
            lhsT16 = top.tile([13, RPAD], dt.float16, tag="lhsT16")
            pd = est.enter_context(tc.tile_pool(name="pd", bufs=1))
            a_own = pd.tile([4, RPAD], dt.float16, tag="a_own")
            nc.vector.tensor_copy(out=a_own[:], in_=cc_sb[:])
            b_own = pd.tile([4, RPAD], dt.float16, tag="b_own")
            nc.vector.tensor_tensor(out=b_own[:], in0=cc_sb[:], in1=a_own[:],
                                    op=OP.subtract)
            na4 = pd.tile([4, RPAD], dt.float16, tag="na4")
            nc.scalar.activation(out=na4[:], in_=a_own[:], func=AF.Copy, scale=-2.0)
            nb4 = pd.tile([4, RPAD], dt.float16, tag="nb4")
            nc.scalar.activation(out=nb4[:], in_=b_own[:], func=AF.Copy, scale=-2.0)
            ones_r = pd.tile([2, RPAD], dt.float16, tag="ones_r")
            nc.vector.memset(ones_r[:], 1.0)
            nc.sync.dma_start(out=lhsT16[0:3, :], in_=na4[0:3, :])
            nc.sync.dma_start(out=lhsT16[3:6, :], in_=na4[0:3, :])
            nc.sync.dma_start(out=lhsT16[6:9, :], in_=nb4[0:3, :])
            nc.sync.dma_start(out=lhsT16[9:10, :], in_=a_own[3:4, :])
            nc.sync.dma_start(out=lhsT16[10:11, :], in_=b_own[3:4, :])
            nc.sync.dma_start(out=lhsT16[11:13, :], in_=ones_r[:])

            # rhs16 rows: [a(3) | b(3) | a(3) | 1 | 1 | sqhi | sqlo]
            rhs_f = pd.tile([4, NCOL], dt.float32, tag="rhs_f")
            nc.vector.memset(rhs_f[:, N:NCOL], 0.0)
            a16 = pd.tile([4, NCOL], dt.float16, tag="a16")
            b16 = pd.tile([4, NCOL], dt.float16, tag="b16")
            vA = lambda t, r0, r1: t[r0:r1, 0:10000].rearrange(
                "p (s c) -> p s c", s=8)[:, :, 0:HALF]
            vB = lambda t, r0, r1: t[r0:r1, 0:10000].rearrange(
                "p (s c) -> p s c", s=8)[:, :, HALF:SHARD]
            # half A (ready while collective B still running)
            for s in range(CORES):
                c0 = s * SHARD
                nc.sync.dma_start(out=rhs_f[0:4, c0:c0 + HALF],
                                  in_=cc_outA[:][s])
            nc.vector.tensor_copy(out=vA(a16, 0, 4), in_=vA(rhs_f, 0, 4))
            nc.vector.tensor_tensor(out=vA(b16, 0, 4), in0=vA(rhs_f, 0, 4),
                                    in1=vA(a16, 0, 4), op=OP.subtract)
            # half B
            for s in range(CORES):
                c0 = s * SHARD
                nc.sync.dma_start(out=rhs_f[0:4, c0 + HALF:c0 + SHARD],
                                  in_=cc_outB[:][s, 0:4, 0:SHARD - HALF])
            nc.vector.tensor_copy(out=vB(a16, 0, 4), in_=vB(rhs_f, 0, 4))
            nc.vector.tensor_tensor(out=vB(b16, 0, 4), in0=vB(rhs_f, 0, 4),
                                    in1=vB(a16, 0, 4), op=OP.subtract)
            # pad cols 10000:10240
            nc.scalar.copy(out=a16[:, N:NCOL], in_=rhs_f[:, N:NCOL])
            nc.scalar.copy(out=b16[:, N:NCOL], in_=rhs_f[:, N:NCOL])
            ones_n = pd.tile([2, NCOL], dt.float16, tag="ones_n")
            nc.vector.memset(ones_n[:], 1.0)
            rhs16 = top.tile([13, NCOL], dt.float16, tag="rhs16")
            nc.sync.dma_start(out=rhs16[0:3, :], in_=a16[0:3, :])
            nc.sync.dma_start(out=rhs16[3:6, :], in_=b16[0:3, :])
            nc.sync.dma_start(out=rhs16[6:9, :], in_=a16[0:3, :])
            nc.sync.dma_start(out=rhs16[9:11, :], in_=ones_n[:])
            nc.sync.dma_start(out=rhs16[11:12, :], in_=a16[3:4, :])
            nc.sync.dma_start(out=rhs16[12:13, :], in_=b16[3:4, :])

            if phases == "ABCD":
                nc.sync.dma_start(out=outD[0:13, 0:NCOL], in_=rhs16[:])
                raise _PhaseDone()

            # ---- phase E: cdist row-block x col-chunk (split-fp16 matmul, fp16 out) ----
            MRG = 4   # psum chunks merged into one output tile/DMA
            with tc.tile_pool(name="pe_d", bufs=3) as pe_d, \
                 tc.tile_pool(name="pe_d2", bufs=3) as pe_d2, \
                 tc.tile_pool(name="pe_ps", bufs=6, space="PSUM") as pe_ps:
                for rb in range(RB):
                    for mg in range(NCOL // (CCH * MRG)):
                        d2t = pe_d2.tile([128, CCH * MRG], dt.float16, tag="d2t")
                        for k in range(MRG):
                            ch = mg * MRG + k
                            dp = pe_ps.tile([128, CCH], dt.float32, tag="dp", space="PSUM")
                            nc.tensor.matmul(
                                dp[:], lhsT16[:, rb * 128:(rb + 1) * 128],
                                rhs16[:, ch * CCH:(ch + 1) * CCH],
                                start=True, stop=True)
                            nc.scalar.activation(out=d2t[:, k * CCH:(k + 1) * CCH],
                                                 in_=dp[:], func=AF.Sqrt,
                                                 bias=eps4_col[:, 0:1])
                        nc.sync.dma_start(
                            out=outD[rb * 128:(rb + 1) * 128,
                                     mg * CCH * MRG:(mg + 1) * CCH * MRG],
                            in_=d2t[:])

    except _PhaseDone:
        pass
    _tc_cm.__exit__(None, None, None)
    nc.compile()
    return nc


def _prep_host(x, edge_index):
    xp = np.zeros((NPAD, FIN), np.float32)
    xp[:N] = np.asarray(x, np.float32)
    xp16 = np.ascontiguousarray(xp.T.astype(np.float16))  # [256, NPAD]

    ei = np.asarray(edge_index)
    src = ei[0].astype(np.int64)
    dst = ei[1].astype(np.int64)

    core = dst // SHARD
    per_core = []
    max_tiles = 0
    for c in range(CORES):
        sel = core == c
        s_c = src[sel]
        d_c = dst[sel]
        loc = d_c - c * SHARD
        blk = loc // 128
        dl = loc - blk * 128
        blocks = []
        for b in range(RB):
            m = blk == b
            blocks.append((s_c[m], dl[m]))
            # tile 0 holds the block's self-loop edges; rest start at tile 1
            max_tiles = max(max_tiles, 1 + (len(blocks[-1][0]) + 127) // 128)
        per_core.append(blocks)

    TB = GC * ((max_tiles + GC - 1) // GC)
    S16 = 8 * TB
    NE = TB * 128

    hidx = np.zeros((CORES, RB, 128, TB), np.int32)
    ohtab = np.zeros((CORES, RB, 128, NE), np.float16)
    ohTtab = np.zeros((CORES, RB, 128, NE), np.float16)
    for c in range(CORES):
        for b in range(RB):
            # tile 0: self loops (edge at partition p has src=dst=block row p)
            rows = c * SHARD + b * 128 + np.arange(128)
            real = rows < N
            crows = np.minimum(rows, N - 1)
            jr = np.arange(128)
            hidx[c, b, jr, 0] = crows.astype(np.int32)
            pr = jr[real]
            ohtab[c, b, pr, pr] = 1.0
            ohTtab[c, b, pr, pr] = 1.0
            # remaining edges from tile 1 on
            s_b, dl_b = per_core[c][b]
            n = len(s_b)
            js = 128 + np.arange(n)
            p = js % 128          # edge partition
            t = js // 128         # edge tile (>= 1)
            hidx[c, b, p, t] = s_b.astype(np.int32)
            sl = dl_b.astype(np.int64)
            ohtab[c, b, p, t * 128 + sl] = 1.0
            ohTtab[c, b, sl, t * 128 + p] = 1.0
    return xp16, hidx, ohtab, ohTtab, TB


def build_in_maps(inputs):
    xp16, hidx, ohtab, ohTtab, TB = _prep_host(inputs["x"], inputs["edge_index"])

    def bc(vec, n, f16=False):
        v = np.asarray(vec, np.float32).reshape(1, n)
        out = np.ascontiguousarray(np.broadcast_to(v, (128, n)).copy())
        return out.astype(np.float16) if f16 else out

    # rhsA: [256 (xfeat, 2 chunks of 128), 262] fp16
    # cols: 0:128 WgT head0 | 128 zero | 129:257 WgT head1 | 257 zero | 258:262 wtil
    Wg = np.asarray(inputs["W_gat"], np.float32)       # [256, 256] rows = H*F out
    att_src = np.asarray(inputs["att_src"], np.float32)  # [2, 128]
    att_dst = np.asarray(inputs["att_dst"], np.float32)
    rhsA = np.zeros((256, 262), np.float32)
    rhsA[:, 0:128] = Wg[0:128, :].T
    rhsA[:, 129:257] = Wg[128:256, :].T
    rhsA[:, 258] = Wg[0:128, :].T @ att_src[0]
    rhsA[:, 259] = Wg[128:256, :].T @ att_src[1]
    rhsA[:, 260] = Wg[0:128, :].T @ att_dst[0]
    rhsA[:, 261] = Wg[128:256, :].T @ att_dst[1]
    rhsA16 = rhsA.astype(np.float16).reshape(2, 128, 262)

    Wa = np.asarray(inputs["Wa"], np.float32)  # [128, 256]
    waT16 = np.ascontiguousarray(Wa.T.astype(np.float16)).reshape(2, 128, 128)
    w1T16 = np.ascontiguousarray(np.asarray(inputs["W1"], np.float32).T.astype(np.float16))
    w2T16 = np.ascontiguousarray(np.asarray(inputs["W2"], np.float32).T.astype(np.float16))
    w3T16 = np.ascontiguousarray(np.asarray(inputs["W3"], np.float32).T.astype(np.float16))

    shared = {
        "xt16": xp16,
        "rhsA": np.ascontiguousarray(rhsA16),
        "waT": waT16, "w1T": w1T16, "w2T": w2T16, "w3T": w3T16,
        "bgat_bc": bc(inputs["b_gat"], 256, f16=True),
        "ba_bc": bc(inputs["ba"], 128),
        "lnaw_bc": bc(inputs["lna_w"], 128), "lnab_bc": bc(inputs["lna_b"], 128),
        "b1_bc": bc(inputs["b1"], 64),
        "ln1w_bc": bc(inputs["ln1_w"], 64), "ln1b_bc": bc(inputs["ln1_b"], 64),
        "b2_bc": bc(inputs["b2"], 32),
        "ln2w_bc": bc(inputs["ln2_w"], 32), "ln2b_bc": bc(inputs["ln2_b"], 32),
        "b3_bc": bc(inputs["b3"], 3),
    }
    in_maps = [
        {**shared, "hidxI": np.ascontiguousarray(hidx[c]),
         "ohtab": np.ascontiguousarray(ohtab[c]),
         "ohTtab": np.ascontiguousarray(ohTtab[c])}
        for c in range(CORES)
    ]
    return in_maps, TB


def kernel(**inputs):
    in_maps, TB = build_in_maps(inputs)

    import os
    phases = os.environ.get("K_PHASES", "ABCDE")
    key = (TB, phases)
    if key not in _BUILD_CACHE:
        _BUILD_CACHE[key] = _build(TB, phases)
    nc = _BUILD_CACHE[key]
    res = run_bass_kernel_spmd(nc, in_maps, core_ids=list(range(CORES)))
    global _LAST_RESULTS
    _LAST_RESULTS = res.results
    out = np.empty((N, N), np.float32)
    for c in range(CORES):
        out[c * SHARD:(c + 1) * SHARD, :] = \
            res.results[c]["outD"][:SHARD, :N].astype(np.float32)
    return out


# revision 25
# speedup vs baseline: 1.1646x; 1.0034x over previous
"""GAT (2-head) + 3x dense/LayerNorm + pairwise-distance kernel for 8 TRN2 NeuronCores.

Strategy: dst-sharded edge processing (one-hot matmul aggregation), replicated
small dense weights, row-block-sharded NxN cdist output.

v3: fp16 htable/gather rows, host-precomputed one-hot tables (no on-device
is_eq), 2-queue GC=12 gathers, whole-x preload, split-fp16 cdist matmuls,
fp16 output.
"""
import sys

import numpy as np

# Environment bootstrap (harness may run from a bare directory).
for _p in ("/root/.axon_site", "/root/.axon_site/_ro/trn_rl_repo",
           "/root/.axon_site/_ro/pypackages", "/opt/trn_rl_repo"):
    if _p not in sys.path:
        sys.path.append(_p)

import concourse.bass as bass
import concourse.bacc as bacc
import concourse.mybir as mybir
import concourse.tile as tile
from concourse.masks import make_identity
from concourse.bass_utils import run_bass_kernel_spmd

dt = mybir.dt
OP = mybir.AluOpType
AF = mybir.ActivationFunctionType

N = 10000
NPAD = 10112          # 79 * 128
NB = 79               # node blocks (phase A)
FIN = 256
F = 128               # per-head GAT dim
H = 2
HROW = 384            # htable row fp16 elems (768B, multiple of 256B)
CORES = 8
SHARD = 1250          # dst nodes per core
RB = 10               # dst blocks per core
RPAD = 1280
CCH = 512             # cdist column chunk
NCOL = 10240          # padded output columns
EPS = 1e-5

_BUILD_CACHE = {}
_LAST_RESULTS = None


GC = 6   # tile groups per dma_gather call (768 idxs = 48 descs/engine)
SUB = 3  # tiles per a_d/exp subchunk


def _build(TB, phases="ABCDE"):
    """Build the 8-core SPMD program. TB = gather tile groups per dst block (mult of GC)."""
    assert TB % GC == 0
    NCH = TB // GC
    S16 = 8 * TB          # idx columns ([128, S16] wrapped int16)

    nc = bacc.Bacc("TRN2", target_bir_lowering=False, debug=False,
                   num_devices=CORES, num_swdge_queues=2)

    def din(name, shape, d=dt.float32):
        return nc.dram_tensor(name, shape, d, kind="ExternalInput").ap()

    xt16 = din("xt16", [FIN, NPAD], dt.float16)
    rhsA = din("rhsA", [2, 128, 262], dt.float16)
    waT = din("waT", [2, 128, 128], dt.float16)
    w1T = din("w1T", [128, 64], dt.float16)
    w2T = din("w2T", [64, 32], dt.float16)
    w3T = din("w3T", [32, 3], dt.float16)
    bgat_bc = din("bgat_bc", [128, 256], dt.float16)
    ba_bc = din("ba_bc", [128, 128])
    lnaw_bc = din("lnaw_bc", [128, 128])
    lnab_bc = din("lnab_bc", [128, 128])
    b1_bc = din("b1_bc", [128, 64])
    ln1w_bc = din("ln1w_bc", [128, 64])
    ln1b_bc = din("ln1b_bc", [128, 64])
    b2_bc = din("b2_bc", [128, 32])
    ln2w_bc = din("ln2w_bc", [128, 32])
    ln2b_bc = din("ln2b_bc", [128, 32])
    b3_bc = din("b3_bc", [128, 3])
    hidxI = din("hidxI", [RB, 128, TB], dt.int32)
    ohtab = din("ohtab", [RB, 128, TB * 128], dt.float16)    # [edge_p, t*128+slot]
    ohTtab = din("ohTtab", [RB, 128, TB * 128], dt.float16)  # [slot_p, t*128+edge]
    outD = nc.dram_tensor("outD", [RPAD, NCOL], dt.float16, kind="ExternalOutput").ap()

    class _PhaseDone(Exception):
        pass

    import contextlib
    try:
        _tc_cm = tile.TileContext(nc)
        tc = _tc_cm.__enter__()
        est = contextlib.ExitStack()
        with est:
            top = est.enter_context(tc.tile_pool(name="top", bufs=1))
            dram = est.enter_context(tc.tile_pool(name="dram", bufs=1, space="DRAM"))

            htable = dram.tile([NPAD, HROW], dt.float16, tag="htable")
            HALF = RPAD // 2
            cc_inA = dram.tile([4, HALF], dt.float32, tag="cc_inA")
            cc_outA = dram.tile([CORES, 4, HALF], dt.float32, tag="cc_outA")
            cc_inB = dram.tile([4, HALF], dt.float32, tag="cc_inB")
            cc_outB = dram.tile([CORES, 4, HALF], dt.float32, tag="cc_outB")

            ident = top.tile([128, 128], dt.float32, tag="ident")
            make_identity(nc, ident[:])
            ident16 = top.tile([128, 128], dt.float16, tag="ident16")
            nc.vector.tensor_copy(out=ident16[:], in_=ident[:])
            eps_col = top.tile([128, 1], dt.float32, tag="eps_col")
            nc.vector.memset(eps_col[:], EPS)
            eps4_col = top.tile([128, 1], dt.float32, tag="eps4_col")
            nc.vector.memset(eps4_col[:], 1e-4)

            # ---- load replicated weights / biases into SBUF ----
            def ldw(name, ap, shape, d=dt.float32):
                t = top.tile(shape, d, tag=name)
                nc.sync.dma_start(out=t[:], in_=ap)
                return t

            rhsA0 = top.tile([128, 262], dt.float16, tag="rhsA0")
            nc.scalar.dma_start(out=rhsA0[:], in_=rhsA[0])
            rhsA1 = top.tile([128, 262], dt.float16, tag="rhsA1")
            nc.scalar.dma_start(out=rhsA1[:], in_=rhsA[1])
            waT0 = ldw("waT0", waT[0], [128, 128], dt.float16)
            waT1 = ldw("waT1", waT[1], [128, 128], dt.float16)
            w1T_sb = ldw("w1T_sb", w1T[:], [128, 64], dt.float16)
            w2T_sb = ldw("w2T_sb", w2T[:], [64, 32], dt.float16)
            w3T_sb = ldw("w3T_sb", w3T[:], [32, 3], dt.float16)
            bgat_sb = ldw("bgat_sb", bgat_bc[:], [128, 256], dt.float16)
            ba_sb = ldw("ba_sb", ba_bc[:], [128, 128])
            lnaw_sb = ldw("lnaw_sb", lnaw_bc[:], [128, 128])
            lnab_sb = ldw("lnab_sb", lnab_bc[:], [128, 128])
            b1_sb = ldw("b1_sb", b1_bc[:], [128, 64])
            ln1w_sb = ldw("ln1w_sb", ln1w_bc[:], [128, 64])
            ln1b_sb = ldw("ln1b_sb", ln1b_bc[:], [128, 64])
            b2_sb = ldw("b2_sb", b2_bc[:], [128, 32])
            ln2w_sb = ldw("ln2w_sb", ln2w_bc[:], [128, 32])
            ln2b_sb = ldw("ln2b_sb", ln2b_bc[:], [128, 32])
            b3_sb = ldw("b3_sb", b3_bc[:], [128, 3])

            # ---- phase A: htable rows [h0 | 1 | h1 | 1 | a_s(2) a_d(2)] fp16 ----
            with tc.tile_pool(name="pa", bufs=1) as pa, \
                 tc.tile_pool(name="pa_ht", bufs=4) as pa_ht, \
                 tc.tile_pool(name="pa_h", bufs=4, space="PSUM") as pa_h:
                xta = pa.tile([128, NPAD], dt.float16, tag="xta")
                xtb = pa.tile([128, NPAD], dt.float16, tag="xtb")
                NH = 1280
                nc.scalar.dma_start(out=xta[:, 0:NH], in_=xt16[0:128, 0:NH])
                nc.scalar.dma_start(out=xtb[:, 0:NH], in_=xt16[128:256, 0:NH])
                nc.scalar.dma_start(out=xta[:, NH:NPAD], in_=xt16[0:128, NH:NPAD])
                nc.scalar.dma_start(out=xtb[:, NH:NPAD], in_=xt16[128:256, NH:NPAD])
                for i in range(NB):
                    r0 = i * 128
                    hp = pa_h.tile([128, 262], dt.float32, tag="hp", space="PSUM")
                    nc.tensor.matmul(hp[:], xta[:, r0:r0 + 128], rhsA0[:],
                                     start=True, stop=False)
                    nc.tensor.matmul(hp[:], xtb[:, r0:r0 + 128], rhsA1[:],
                                     start=False, stop=True)
                    ht = pa_ht.tile([128, 262], dt.float16, tag="ht")
                    if i % 2 == 0:
                        nc.scalar.copy(out=ht[:], in_=hp[:])
                    else:
                        nc.vector.tensor_copy(out=ht[:], in_=hp[:])
                    nc.gpsimd.memset(ht[:, 128:129], 1.0)
                    nc.gpsimd.memset(ht[:, 257:258], 1.0)
                    nc.sync.dma_start(out=htable[r0:r0 + 128, 0:262], in_=ht[:])

            if phases == "A":
                dbg = top.tile([128, 262], dt.float16, tag="dbgA")
                for i in range(RB):
                    nc.sync.dma_start(out=dbg[:], in_=htable[i * 128:(i + 1) * 128, 0:262])
                    nc.sync.dma_start(out=outD[i * 128:(i + 1) * 128, 0:262], in_=dbg[:])
                raise _PhaseDone()

            # ---- phase B: GAT aggregation per dst block ----
            xg_pool = est.enter_context(tc.tile_pool(name="xg", bufs=1))
            xgs = []
            with tc.tile_pool(name="pb_idx", bufs=2) as pb_idx, \
                 tc.tile_pool(name="pb_oht", bufs=3) as pb_oht, \
                 tc.tile_pool(name="pb_g", bufs=6) as pb_g, \
                 tc.tile_pool(name="pb_ad", bufs=2) as pb_ad, \
                 tc.tile_pool(name="pb_ex", bufs=4) as pb_ex, \
                 tc.tile_pool(name="pb_rhs", bufs=6) as pb_rhs, \
                 tc.tile_pool(name="pb_ep", bufs=2) as pb_ep, \
                 tc.tile_pool(name="pb_ps", bufs=2, space="PSUM") as pb_ps, \
                 tc.tile_pool(name="pb_adp", bufs=4, space="PSUM") as pb_adp:
                for _slot in range(6):
                    gz = pb_g.tile([128, GC, HROW], dt.float16, tag="g")
                    nc.vector.memset(gz[:], 0.0)
                for b in range(RB):
                    hix = pb_idx.tile([128, TB], dt.int32, tag="hix")
                    nc.sync.dma_start(out=hix[:], in_=hidxI[b])
                    oh_sb = pb_oht.tile([128, TB * 128], dt.float16, tag="oh_sb")
                    nc.sync.dma_start(out=oh_sb[:], in_=ohtab[b])
                    ohT_sb = pb_oht.tile([128, TB * 128], dt.float16, tag="ohT_sb")
                    nc.sync.dma_start(out=ohT_sb[:], in_=ohTtab[b])

                    adblk = pb_ad.tile([128, 2], dt.float16, tag="adblk")
                    ps = pb_ps.tile([128, 258], dt.float32, tag="ps", space="PSUM")

                    for c in range(NCH):
                        g = pb_g.tile([128, GC, HROW], dt.float16, tag="g")
                        for tl in range(GC):
                            nc.gpsimd.indirect_dma_start(
                                out=g[:, tl, :], out_offset=None,
                                in_=htable[:],
                                in_offset=bass.IndirectOffsetOnAxis(
                                    ap=hix[:, c * GC + tl:c * GC + tl + 1], axis=0))
                        if c == 0:
                            # tile 0 rows are this block's own dst rows (self loops)
                            nc.vector.tensor_copy(out=adblk[:], in_=g[:, 0, 260:262])
                        for hs in range(GC // SUB):
                            t0 = c * GC + hs * SUB
                            adps = pb_adp.tile([128, SUB, 2], dt.float32, tag="adps",
                                               space="PSUM")
                            for k in range(SUB):
                                t = t0 + k
                                nc.tensor.matmul(adps[:, k, :],
                                                 ohT_sb[:, t * 128:(t + 1) * 128],
                                                 adblk[:], start=True, stop=True)
                            # e = a_s[src] + a_d[dst]; exv = max(exp(e), exp(0.2e))
                            exr = pb_ex.tile([128, SUB, 2], dt.float32, tag="exr")
                            nc.vector.tensor_tensor(
                                out=exr[:], in0=adps[:],
                                in1=g[:, hs * SUB:(hs + 1) * SUB, 258:260], op=OP.add)
                            exn = pb_ex.tile([128, SUB, 2], dt.float32, tag="exn")
                            nc.scalar.activation(out=exn[:], in_=exr[:], func=AF.Exp,
                                                 scale=0.2)
                            exp_ = pb_ex.tile([128, SUB, 2], dt.float32, tag="exp_")
                            nc.scalar.activation(out=exp_[:], in_=exr[:], func=AF.Exp)
                            exv = pb_ex.tile([128, SUB, 2], dt.float32, tag="exv")
                            nc.vector.tensor_tensor(out=exv[:], in0=exp_[:], in1=exn[:],
                                                    op=OP.max)
                            for k in range(SUB):
                                t = t0 + k
                                tl = hs * SUB + k
                                rhs = pb_rhs.tile([128, 258], dt.float16, tag="rhs")
                                if t % 2 == 0:
                                    nc.scalar.activation(out=rhs[:, 0:129],
                                                         in_=g[:, tl, 0:129],
                                                         func=AF.Copy,
                                                         scale=exv[:, k, 0:1])
                                    nc.vector.tensor_scalar(
                                        out=rhs[:, 129:258], in0=g[:, tl, 129:258],
                                        scalar1=exv[:, k, 1:2], scalar2=None, op0=OP.mult)
                                else:
                                    nc.vector.tensor_scalar(
                                        out=rhs[:, 0:129], in0=g[:, tl, 0:129],
                                        scalar1=exv[:, k, 0:1], scalar2=None, op0=OP.mult)
                                    nc.scalar.activation(out=rhs[:, 129:258],
                                                         in_=g[:, tl, 129:258],
                                                         func=AF.Copy,
                                                         scale=exv[:, k, 1:2])
                                nc.tensor.matmul(ps[:],
                                                 oh_sb[:, t * 128:(t + 1) * 128],
                                                 rhs[:],
                                                 start=(t == 0), stop=(t == TB - 1))

                    # epilogue: normalize, +b_gat, lrelu(0.01) -> fp16 xg
                    rec0 = pb_ep.tile([128, 1], dt.float32, tag="rec0")
                    nc.vector.reciprocal(rec0[:], ps[:, 128:129])
                    rec1 = pb_ep.tile([128, 1], dt.float32, tag="rec1")
                    nc.vector.reciprocal(rec1[:], ps[:, 257:258])
                    xg = xg_pool.tile([128, 256], dt.float16, tag=f"xg{b}")
                    nc.scalar.activation(out=xg[:, 0:128], in_=ps[:, 0:128],
                                         func=AF.Copy, scale=rec0[:])
                    nc.scalar.activation(out=xg[:, 128:256], in_=ps[:, 129:257],
                                         func=AF.Copy, scale=rec1[:])
                    nc.vector.tensor_tensor(out=xg[:], in0=xg[:], in1=bgat_sb[:], op=OP.add)
                    ng = pb_ep.tile([128, 256], dt.float16, tag="ng")
                    nc.vector.tensor_scalar(out=ng[:], in0=xg[:], scalar1=0.0,
                                            scalar2=0.01, op0=OP.min, op1=OP.mult)
                    nc.vector.scalar_tensor_tensor(out=xg[:], in0=xg[:], scalar=0.0,
                                                   in1=ng[:], op0=OP.max, op1=OP.add)
                    xgs.append(xg)

            if phases == "AB":
                for b in range(RB):
                    nc.sync.dma_start(out=outD[b * 128:(b + 1) * 128, 0:256], in_=xgs[b][:])
                raise _PhaseDone()

            # ---- phase C: dense + LN on own shard (stage-parallel across blocks) ----
            cc_sb = top.tile([4, RPAD], dt.float32, tag="cc_sb")
            with tc.tile_pool(name="pc", bufs=12) as pc, \
                 tc.tile_pool(name="pc_ps", bufs=2, space="PSUM") as pc_ps, \
                 tc.tile_pool(name="pc_mm", bufs=4, space="PSUM") as pc_mm:

                def transpose16(xin, pdim, fdim):
                    # xin fp16 [pdim, fdim] -> sbuf fp16 [fdim, pdim]
                    p = pc_ps.tile([128, 128], dt.float16, tag="tpp16", space="PSUM")
                    nc.tensor.transpose(out=p[:fdim, 0:pdim], in_=xin,
                                        identity=ident16[:pdim, :pdim])
                    s = pc.tile([fdim, pdim], dt.float16, tag=f"tt{fdim}_{pdim}")
                    nc.scalar.copy(out=s[:], in_=p[:fdim, 0:pdim])
                    return s

                def c_chain(b):
                    x0 = xgs[b]
                    xt0 = transpose16(x0[:, 0:128], 128, 128)
                    yield
                    xt1 = transpose16(x0[:, 128:256], 128, 128)
                    yield
                    pA = pc_mm.tile([128, 128], dt.float32, tag="mm", space="PSUM")
                    nc.tensor.matmul(pA[:], xt0[:], waT0[:], start=True, stop=False)
                    nc.tensor.matmul(pA[:], xt1[:], waT1[:], start=False, stop=True)
                    yield
                    x1 = yield from ln_lrelu(b, pA[:], 128, ba_sb, lnaw_sb, lnab_sb)
                    x1t = transpose16(x1[:], 128, 128)
                    yield
                    p1 = pc_mm.tile([128, 64], dt.float32, tag="mm", space="PSUM")
                    nc.tensor.matmul(p1[:], x1t[:], w1T_sb[:], start=True, stop=True)
                    yield
                    x2 = yield from ln_lrelu(b, p1[:], 64, b1_sb, ln1w_sb, ln1b_sb)
                    x2t = transpose16(x2[:], 128, 64)
                    yield
                    p2 = pc_mm.tile([128, 32], dt.float32, tag="mm", space="PSUM")
                    nc.tensor.matmul(p2[:], x2t[:], w2T_sb[:], start=True, stop=True)
                    yield
                    x3 = yield from ln_lrelu(b, p2[:], 32, b2_sb, ln2w_sb, ln2b_sb)
                    x3t = transpose16(x3[:], 128, 32)
                    yield
                    p3 = pc_mm.tile([128, 3], dt.float32, tag="mm", space="PSUM")
                    nc.tensor.matmul(p3[:], x3t[:], w3T_sb[:], start=True, stop=True)
                    yield
                    y3 = pc.tile([128, 4], dt.float32, tag="y3")
                    nc.vector.tensor_tensor(out=y3[:, 0:3], in0=p3[:], in1=b3_sb[:], op=OP.add)
                    scr3 = pc.tile([128, 3], dt.float32, tag="scr3")
                    nc.scalar.activation(out=scr3[:], in_=y3[:, 0:3], func=AF.Square,
                                         accum_out=y3[:, 3:4])
                    yield
                    h3p = pc_ps.tile([128, 128], dt.float32, tag="tpp", space="PSUM")
                    nc.tensor.transpose(out=h3p[:4, 0:128], in_=y3[:], identity=ident[:])
                    nc.scalar.copy(out=cc_sb[:, b * 128:(b + 1) * 128], in_=h3p[:4, 0:128])

                def ln_lrelu(b, xin, fdim, bias_bc, w_bc, b_bc):
                    # y = xin + bias; u = LN(y)*w + b; return lrelu001(u) fp16
                    y = pc.tile([128, fdim], dt.float32, tag=f"y{fdim}")
                    nc.vector.tensor_tensor(out=y[:], in0=xin, in1=bias_bc[:], op=OP.add)
                    yield
                    scr = pc.tile([128, fdim], dt.float32, tag=f"scr{fdim}")
                    msum = pc.tile([128, 1], dt.float32, tag="msum")
                    nc.scalar.activation(out=scr[:], in_=y[:], func=AF.Copy,
                                         accum_out=msum[:])
                    sqs = pc.tile([128, 1], dt.float32, tag="sqs")
                    nc.scalar.activation(out=scr[:], in_=y[:], func=AF.Square,
                                         accum_out=sqs[:])
                    yield
                    mean = pc.tile([128, 1], dt.float32, tag="mean")
                    nc.vector.tensor_scalar(out=mean[:], in0=msum[:], scalar1=1.0 / fdim,
                                            scalar2=None, op0=OP.mult)
                    var = pc.tile([128, 1], dt.float32, tag="var")
                    nc.vector.tensor_scalar(out=var[:], in0=sqs[:], scalar1=1.0 / fdim,
                                            scalar2=None, op0=OP.mult)
                    m2 = pc.tile([128, 1], dt.float32, tag="m2")
                    nc.vector.tensor_scalar(out=m2[:], in0=mean[:], scalar1=mean[:, 0:1],
                                            scalar2=None, op0=OP.mult)
                    nc.vector.tensor_tensor(out=var[:], in0=var[:], in1=m2[:], op=OP.subtract)
                    sd = pc.tile([128, 1], dt.float32, tag="sd")
                    nc.scalar.activation(out=sd[:], in_=var[:], func=AF.Sqrt,
                                         bias=eps_col[:, 0:1])
                    rstd = pc.tile([128, 1], dt.float32, tag="rstd")
                    nc.vector.reciprocal(rstd[:], sd[:])
                    yield
                    u = pc.tile([128, fdim], dt.float32, tag=f"u{fdim}")
                    nc.vector.scalar_tensor_tensor(out=u[:], in0=y[:], scalar=mean[:, 0:1],
                                                   in1=w_bc[:], op0=OP.subtract, op1=OP.mult)
                    nc.vector.scalar_tensor_tensor(out=u[:], in0=u[:], scalar=rstd[:, 0:1],
                                                   in1=b_bc[:], op0=OP.mult, op1=OP.add)
                    yield
                    ngt = pc.tile([128, fdim], dt.float32, tag=f"ng{fdim}")
                    nc.vector.tensor_scalar(out=ngt[:], in0=u[:], scalar1=0.0,
                                            scalar2=0.01, op0=OP.min, op1=OP.mult)
                    u16 = pc.tile([128, fdim], dt.float16, tag=f"u16_{fdim}")
                    nc.vector.scalar_tensor_tensor(out=u16[:], in0=u[:], scalar=0.0,
                                                   in1=ngt[:], op0=OP.max, op1=OP.add)
                    yield
                    return u16

                gens = [c_chain(b) for b in range(RB)]

                def run_gens(idxs):
                    done = {b: False for b in idxs}
                    while not all(done.values()):
                        for b in idxs:
                            if not done[b]:
                                try:
                                    next(gens[b])
                                except StopIteration:
                                    done[b] = True

                run_gens(range(RB // 2))
                nc.sync.dma_start(out=cc_inA[:], in_=cc_sb[:, 0:HALF])
                nc.gpsimd.collective_compute(
                    "AllGather", OP.bypass, replica_groups=[list(range(CORES))],
                    ins=[cc_inA[:].opt()], outs=[cc_outA[:].opt()])
                run_gens(range(RB // 2, RB))
                nc.sync.dma_start(out=cc_inB[:], in_=cc_sb[:, HALF:RPAD])
                nc.gpsimd.collective_compute(
                    "AllGather", OP.bypass, replica_groups=[list(range(CORES))],
                    ins=[cc_inB[:].opt()], outs=[cc_outB[:].opt()])

            if phases == "ABC":
                dbg16 = top.tile([4, RPAD], dt.float16, tag="dbgc")
                nc.vector.tensor_copy(out=dbg16[:], in_=cc_sb[:])
                nc.sync.dma_start(out=outD[0:4, 0:RPAD], in_=dbg16[:])
                raise _PhaseDone()

            # ---- phase D: build split-fp16 cdist operands ----
            # lhsT16 rows: [-2a(3) | -2a(3) | -2b(3) | sqhi | sqlo | 1 | 1]
            # (compute in partition-0 tiles, assemble via sbuf-to-sbuf DMA)
            lhsT16 = top.tile([13, RPAD], dt.float16, tag="lhsT16")
            pd = est.enter_context(tc.tile_pool(name="pd", bufs=1))
            a_own = pd.tile([4, RPAD], dt.float16, tag="a_own")
            nc.vector.tensor_copy(out=a_own[:], in_=cc_sb[:])
            b_own = pd.tile([4, RPAD], dt.float16, tag="b_own")
            nc.vector.tensor_tensor(out=b_own[:], in0=cc_sb[:], in1=a_own[:],
                                    op=OP.subtract)
            na4 = pd.tile([4, RPAD], dt.float16, tag="na4")
            nc.scalar.activation(out=na4[:], in_=a_own[:], func=AF.Copy, scale=-2.0)
            nb4 = pd.tile([4, RPAD], dt.float16, tag="nb4")
            nc.scalar.activation(out=nb4[:], in_=b_own[:], func=AF.Copy, scale=-2.0)
            ones_r = pd.tile([2, RPAD], dt.float16, tag="ones_r")
            nc.vector.memset(ones_r[:], 1.0)
            nc.sync.dma_start(out=lhsT16[0:3, :], in_=na4[0:3, :])
            nc.sync.dma_start(out=lhsT16[3:6, :], in_=na4[0:3, :])
            nc.sync.dma_start(out=lhsT16[6:9, :], in_=nb4[0:3, :])
            nc.sync.dma_start(out=lhsT16[9:10, :], in_=a_own[3:4, :])
            nc.sync.dma_start(out=lhsT16[10:11, :], in_=b_own[3:4, :])
            nc.sync.dma_start(out=lhsT16[11:13, :], in_=ones_r[:])

            # rhs16 rows: [a(3) | b(3) | a(3) | 1 | 1 | sqhi | sqlo]
            rhs_f = pd.tile([4, NCOL], dt.float32, tag="rhs_f")
            nc.vector.memset(rhs_f[:, N:NCOL], 0.0)
            a16 = pd.tile([4, NCOL], dt.float16, tag="a16")
            b16 = pd.tile([4, NCOL], dt.float16, tag="b16")
            vA = lambda t, r0, r1: t[r0:r1, 0:10000].rearrange(
                "p (s c) -> p s c", s=8)[:, :, 0:HALF]
            vB = lambda t, r0, r1: t[r0:r1, 0:10000].rearrange(
                "p (s c) -> p s c", s=8)[:, :, HALF:SHARD]
            # half A (ready while collective B still running)
            for s in range(CORES):
                c0 = s * SHARD
                nc.sync.dma_start(out=rhs_f[0:4, c0:c0 + HALF],
                                  in_=cc_outA[:][s])
            nc.vector.tensor_copy(out=vA(a16, 0, 4), in_=vA(rhs_f, 0, 4))
            nc.vector.tensor_tensor(out=vA(b16, 0, 4), in0=vA(rhs_f, 0, 4),
                                    in1=vA(a16, 0, 4), op=OP.subtract)
            # half B
            for s in range(CORES):
                c0 = s * SHARD
                nc.sync.dma_start(out=rhs_f[0:4, c0 + HALF:c0 + SHARD],
                                  in_=cc_outB[:][s, 0:4, 0:SHARD - HALF])
            nc.vector.tensor_copy(out=vB(a16, 0, 4), in_=vB(rhs_f, 0, 4))
            nc.vector.tensor_tensor(out=vB(b16, 0, 4), in0=vB(rhs_f, 0, 4),
                                    in1=vB(a16, 0, 4), op=OP.subtract)
            # pad cols 10000:10240
            nc.scalar.copy(out=a16[:, N:NCOL], in_=rhs_f[:, N:NCOL])
            nc.scalar.copy(out=b16[:, N:NCOL], in_=rhs_f[:, N:NCOL])
            ones_n = pd.tile([2, NCOL], dt.float16, tag="ones_n")
            nc.vector.memset(ones_n[:], 1.0)
            rhs16 = top.tile([13, NCOL], dt.float16, tag="rhs16")
            nc.sync.dma_start(out=rhs16[0:3, :], in_=a16[0:3, :])
            nc.sync.dma_start(out=rhs16[3:6, :], in_=b16[0:3, :])
            nc.sync.dma_start(out=rhs16[6:9, :], in_=a16[0:3, :])
            nc.sync.dma_start(out=rhs16[9:11, :], in_=ones_n[:])
            nc.sync.dma_start(out=rhs16[11:12, :], in_=a16[3:4, :])
            nc.sync.dma_start(out=rhs16[12:13, :], in_=b16[3:4, :])

            if phases == "ABCD":
                nc.sync.dma_start(out=outD[0:13, 0:NCOL], in_=rhs16[:])
                raise _PhaseDone()

            # ---- phase E: cdist row-block x col-chunk (split-fp16 matmul, fp16 out) ----
            MRG = 5   # psum chunks merged into one output tile/DMA
            with tc.tile_pool(name="pe_d", bufs=3) as pe_d, \
                 tc.tile_pool(name="pe_d2", bufs=3) as pe_d2, \
                 tc.tile_pool(name="pe_ps", bufs=6, space="PSUM") as pe_ps:
                for rb in range(RB):
                    for mg in range(NCOL // (CCH * MRG)):
                        d2t = pe_d2.tile([128, CCH * MRG], dt.float16, tag="d2t")
                        for k in range(MRG):
                            ch = mg * MRG + k
                            dp = pe_ps.tile([128, CCH], dt.float32, tag="dp", space="PSUM")
                            nc.tensor.matmul(
                                dp[:], lhsT16[:, rb * 128:(rb + 1) * 128],
                                rhs16[:, ch * CCH:(ch + 1) * CCH],
                                start=True, stop=True)
                            nc.scalar.activation(out=d2t[:, k * CCH:(k + 1) * CCH],
                                                 in_=dp[:], func=AF.Sqrt,
                                                 bias=eps4_col[:, 0:1])
                        nc.sync.dma_start(
                            out=outD[rb * 128:(rb + 1) * 128,
                                     mg * CCH * MRG:(mg + 1) * CCH * MRG],
                            in_=d2t[:])

    except _PhaseDone:
        pass
    _tc_cm.__exit__(None, None, None)
    nc.compile()
    return nc


def _prep_host(x, edge_index):
    xp = np.zeros((NPAD, FIN), np.float32)
    xp[:N] = np.asarray(x, np.float32)
    xp16 = np.ascontiguousarray(xp.T.astype(np.float16))  # [256, NPAD]

    ei = np.asarray(edge_index)
    src = ei[0].astype(np.int64)
    dst = ei[1].astype(np.int64)

    core = dst // SHARD
    per_core = []
    max_tiles = 0
    for c in range(CORES):
        sel = core == c
        s_c = src[sel]
        d_c = dst[sel]
        loc = d_c - c * SHARD
        blk = loc // 128
        dl = loc - blk * 128
        blocks = []
        for b in range(RB):
            m = blk == b
            blocks.append((s_c[m], dl[m]))
            # tile 0 holds the block's self-loop edges; rest start at tile 1
            max_tiles = max(max_tiles, 1 + (len(blocks[-1][0]) + 127) // 128)
        per_core.append(blocks)

    TB = GC * ((max_tiles + GC - 1) // GC)
    S16 = 8 * TB
    NE = TB * 128

    hidx = np.zeros((CORES, RB, 128, TB), np.int32)
    ohtab = np.zeros((CORES, RB, 128, NE), np.float16)
    ohTtab = np.zeros((CORES, RB, 128, NE), np.float16)
    for c in range(CORES):
        for b in range(RB):
            # tile 0: self loops (edge at partition p has src=dst=block row p)
            rows = c * SHARD + b * 128 + np.arange(128)
            real = rows < N
            crows = np.minimum(rows, N - 1)
            jr = np.arange(128)
            hidx[c, b, jr, 0] = crows.astype(np.int32)
            pr = jr[real]
            ohtab[c, b, pr, pr] = 1.0
            ohTtab[c, b, pr, pr] = 1.0
            # remaining edges from tile 1 on
            s_b, dl_b = per_core[c][b]
            n = len(s_b)
            js = 128 + np.arange(n)
            p = js % 128          # edge partition
            t = js // 128         # edge tile (>= 1)
            hidx[c, b, p, t] = s_b.astype(np.int32)
            sl = dl_b.astype(np.int64)
            ohtab[c, b, p, t * 128 + sl] = 1.0
            ohTtab[c, b, sl, t * 128 + p] = 1.0
    return xp16, hidx, ohtab, ohTtab, TB


def build_in_maps(inputs):
    xp16, hidx, ohtab, ohTtab, TB = _prep_host(inputs["x"], inputs["edge_index"])

    def bc(vec, n, f16=False):
        v = np.asarray(vec, np.float32).reshape(1, n)
        out = np.ascontiguousarray(np.broadcast_to(v, (128, n)).copy())
        return out.astype(np.float16) if f16 else out

    # rhsA: [256 (xfeat, 2 chunks of 128), 262] fp16
    # cols: 0:128 WgT head0 | 128 zero | 129:257 WgT head1 | 257 zero | 258:262 wtil
    Wg = np.asarray(inputs["W_gat"], np.float32)       # [256, 256] rows = H*F out
    att_src = np.asarray(inputs["att_src"], np.float32)  # [2, 128]
    att_dst = np.asarray(inputs["att_dst"], np.float32)
    rhsA = np.zeros((256, 262), np.float32)
    rhsA[:, 0:128] = Wg[0:128, :].T
    rhsA[:, 129:257] = Wg[128:256, :].T
    rhsA[:, 258] = Wg[0:128, :].T @ att_src[0]
    rhsA[:, 259] = Wg[128:256, :].T @ att_src[1]
    rhsA[:, 260] = Wg[0:128, :].T @ att_dst[0]
    rhsA[:, 261] = Wg[128:256, :].T @ att_dst[1]
    rhsA16 = rhsA.astype(np.float16).reshape(2, 128, 262)

    Wa = np.asarray(inputs["Wa"], np.float32)  # [128, 256]
    waT16 = np.ascontiguousarray(Wa.T.astype(np.float16)).reshape(2, 128, 128)
    w1T16 = np.ascontiguousarray(np.asarray(inputs["W1"], np.float32).T.astype(np.float16))
    w2T16 = np.ascontiguousarray(np.asarray(inputs["W2"], np.float32).T.astype(np.float16))
    w3T16 = np.ascontiguousarray(np.asarray(inputs["W3"], np.float32).T.astype(np.float16))

    shared = {
        "xt16": xp16,
        "rhsA": np.ascontiguousarray(rhsA16),
        "waT": waT16, "w1T": w1T16, "w2T": w2T16, "w3T": w3T16,
        "bgat_bc": bc(inputs["b_gat"], 256, f16=True),
        "ba_bc": bc(inputs["ba"], 128),
        "lnaw_bc": bc(inputs["lna_w"], 128), "lnab_bc": bc(inputs["lna_b"], 128),
        "b1_bc": bc(inputs["b1"], 64),
        "ln1w_bc": bc(inputs["ln1_w"], 64), "ln1b_bc": bc(inputs["ln1_b"], 64),
        "b2_bc": bc(inputs["b2"], 32),
        "ln2w_bc": bc(inputs["ln2_w"], 32), "ln2b_bc": bc(inputs["ln2_b"], 32),
        "b3_bc": bc(inputs["b3"], 3),
    }
    in_maps = [
        {**shared, "hidxI": np.ascontiguousarray(hidx[c]),
         "ohtab": np.ascontiguousarray(ohtab[c]),
         "ohTtab": np.ascontiguousarray(ohTtab[c])}
        for c in range(CORES)
    ]
    return in_maps, TB


def kernel(**inputs):
    in_maps, TB = build_in_maps(inputs)

    import os
    phases = os.environ.get("K_PHASES", "ABCDE")
    key = (TB, phases)
    if key not in _BUILD_CACHE:
        _BUILD_CACHE[key] = _build(TB, phases)
    nc = _BUILD_CACHE[key]
    res = run_bass_kernel_spmd(nc, in_maps, core_ids=list(range(CORES)))
    global _LAST_RESULTS
    _LAST_RESULTS = res.results
    out = np.empty((N, N), np.float32)
    for c in range(CORES):
        out[c * SHARD:(c + 1) * SHARD, :] = \
            res.results[c]["outD"][:SHARD, :N].astype(np.float32)
    return out


# revision 26
# speedup vs baseline: 1.1647x; 1.0001x over previous
"""GAT (2-head) + 3x dense/LayerNorm + pairwise-distance kernel for 8 TRN2 NeuronCores.

Strategy: dst-sharded edge processing (one-hot matmul aggregation), replicated
small dense weights, row-block-sharded NxN cdist output.

v3: fp16 htable/gather rows, host-precomputed one-hot tables (no on-device
is_eq), 2-queue GC=12 gathers, whole-x preload, split-fp16 cdist matmuls,
fp16 output.
"""
import sys

import numpy as np

# Environment bootstrap (harness may run from a bare directory).
for _p in ("/root/.axon_site", "/root/.axon_site/_ro/trn_rl_repo",
           "/root/.axon_site/_ro/pypackages", "/opt/trn_rl_repo"):
    if _p not in sys.path:
        sys.path.append(_p)

import concourse.bass as bass
import concourse.bacc as bacc
import concourse.mybir as mybir
import concourse.tile as tile
from concourse.masks import make_identity
from concourse.bass_utils import run_bass_kernel_spmd

dt = mybir.dt
OP = mybir.AluOpType
AF = mybir.ActivationFunctionType

N = 10000
NPAD = 10112          # 79 * 128
NB = 79               # node blocks (phase A)
FIN = 256
F = 128               # per-head GAT dim
H = 2
HROW = 384            # htable row fp16 elems (768B, multiple of 256B)
CORES = 8
SHARD = 1250          # dst nodes per core
RB = 10               # dst blocks per core
RPAD = 1280
CCH = 512             # cdist column chunk
NCOL = 10240          # padded output columns
EPS = 1e-5

_BUILD_CACHE = {}
_LAST_RESULTS = None


GC = 6   # tile groups per dma_gather call (768 idxs = 48 descs/engine)
SUB = 3  # tiles per a_d/exp subchunk


def _build(TB, phases="ABCDE"):
    """Build the 8-core SPMD program. TB = gather tile groups per dst block (mult of GC)."""
    assert TB % GC == 0
    NCH = TB // GC
    S16 = 8 * TB          # idx columns ([128, S16] wrapped int16)

    nc = bacc.Bacc("TRN2", target_bir_lowering=False, debug=False,
                   num_devices=CORES, num_swdge_queues=2)

    def din(name, shape, d=dt.float32):
        return nc.dram_tensor(name, shape, d, kind="ExternalInput").ap()

    xt16 = din("xt16", [FIN, NPAD], dt.float16)
    rhsA = din("rhsA", [2, 128, 262], dt.float16)
    waT = din("waT", [2, 128, 128], dt.float16)
    w1T = din("w1T", [128, 64], dt.float16)
    w2T = din("w2T", [64, 32], dt.float16)
    w3T = din("w3T", [32, 3], dt.float16)
    bgat_bc = din("bgat_bc", [128, 256], dt.float16)
    ba_bc = din("ba_bc", [128, 128])
    lnaw_bc = din("lnaw_bc", [128, 128])
    lnab_bc = din("lnab_bc", [128, 128])
    b1_bc = din("b1_bc", [128, 64])
    ln1w_bc = din("ln1w_bc", [128, 64])
    ln1b_bc = din("ln1b_bc", [128, 64])
    b2_bc = din("b2_bc", [128, 32])
    ln2w_bc = din("ln2w_bc", [128, 32])
    ln2b_bc = din("ln2b_bc", [128, 32])
    b3_bc = din("b3_bc", [128, 3])
    hidxI = din("hidxI", [RB, 128, TB], dt.int32)
    ohtab = din("ohtab", [RB, 128, TB * 128], dt.float16)    # [edge_p, t*128+slot]
    ohTtab = din("ohTtab", [RB, 128, TB * 128], dt.float16)  # [slot_p, t*128+edge]
    outD = nc.dram_tensor("outD", [RPAD, NCOL], dt.float16, kind="ExternalOutput").ap()

    class _PhaseDone(Exception):
        pass

    import contextlib
    try:
        _tc_cm = tile.TileContext(nc)
        tc = _tc_cm.__enter__()
        est = contextlib.ExitStack()
        with est:
            top = est.enter_context(tc.tile_pool(name="top", bufs=1))
            dram = est.enter_context(tc.tile_pool(name="dram", bufs=1, space="DRAM"))

            htable = dram.tile([NPAD, HROW], dt.float16, tag="htable")
            HALF = RPAD // 2
            cc_inA = dram.tile([4, HALF], dt.float32, tag="cc_inA")
            cc_outA = dram.tile([CORES, 4, HALF], dt.float32, tag="cc_outA")
            cc_inB = dram.tile([4, HALF], dt.float32, tag="cc_inB")
            cc_outB = dram.tile([CORES, 4, HALF], dt.float32, tag="cc_outB")

            ident = top.tile([128, 128], dt.float32, tag="ident")
            make_identity(nc, ident[:])
            ident16 = top.tile([128, 128], dt.float16, tag="ident16")
            nc.vector.tensor_copy(out=ident16[:], in_=ident[:])
            eps_col = top.tile([128, 1], dt.float32, tag="eps_col")
            nc.vector.memset(eps_col[:], EPS)
            eps4_col = top.tile([128, 1], dt.float32, tag="eps4_col")
            nc.vector.memset(eps4_col[:], 1e-4)

            # ---- load replicated weights / biases into SBUF ----
            def ldw(name, ap, shape, d=dt.float32):
                t = top.tile(shape, d, tag=name)
                nc.sync.dma_start(out=t[:], in_=ap)
                return t

            rhsA0 = top.tile([128, 262], dt.float16, tag="rhsA0")
            nc.scalar.dma_start(out=rhsA0[:], in_=rhsA[0])
            rhsA1 = top.tile([128, 262], dt.float16, tag="rhsA1")
            nc.scalar.dma_start(out=rhsA1[:], in_=rhsA[1])
            waT0 = ldw("waT0", waT[0], [128, 128], dt.float16)
            waT1 = ldw("waT1", waT[1], [128, 128], dt.float16)
            w1T_sb = ldw("w1T_sb", w1T[:], [128, 64], dt.float16)
            w2T_sb = ldw("w2T_sb", w2T[:], [64, 32], dt.float16)
            w3T_sb = ldw("w3T_sb", w3T[:], [32, 3], dt.float16)
            bgat_sb = ldw("bgat_sb", bgat_bc[:], [128, 256], dt.float16)
            ba_sb = ldw("ba_sb", ba_bc[:], [128, 128])
            lnaw_sb = ldw("lnaw_sb", lnaw_bc[:], [128, 128])
            lnab_sb = ldw("lnab_sb", lnab_bc[:], [128, 128])
            b1_sb = ldw("b1_sb", b1_bc[:], [128, 64])
            ln1w_sb = ldw("ln1w_sb", ln1w_bc[:], [128, 64])
            ln1b_sb = ldw("ln1b_sb", ln1b_bc[:], [128, 64])
            b2_sb = ldw("b2_sb", b2_bc[:], [128, 32])
            ln2w_sb = ldw("ln2w_sb", ln2w_bc[:], [128, 32])
            ln2b_sb = ldw("ln2b_sb", ln2b_bc[:], [128, 32])
            b3_sb = ldw("b3_sb", b3_bc[:], [128, 3])

            # ---- phase A: htable rows [h0 | 1 | h1 | 1 | a_s(2) a_d(2)] fp16 ----
            with tc.tile_pool(name="pa", bufs=1) as pa, \
                 tc.tile_pool(name="pa_ht", bufs=4) as pa_ht, \
                 tc.tile_pool(name="pa_h", bufs=4, space="PSUM") as pa_h:
                xta = pa.tile([128, NPAD], dt.float16, tag="xta")
                xtb = pa.tile([128, NPAD], dt.float16, tag="xtb")
                NH = 1280
                nc.scalar.dma_start(out=xta[:, 0:NH], in_=xt16[0:128, 0:NH])
                nc.scalar.dma_start(out=xtb[:, 0:NH], in_=xt16[128:256, 0:NH])
                nc.scalar.dma_start(out=xta[:, NH:NPAD], in_=xt16[0:128, NH:NPAD])
                nc.scalar.dma_start(out=xtb[:, NH:NPAD], in_=xt16[128:256, NH:NPAD])
                for i in range(NB):
                    r0 = i * 128
                    hp = pa_h.tile([128, 262], dt.float32, tag="hp", space="PSUM")
                    nc.tensor.matmul(hp[:], xta[:, r0:r0 + 128], rhsA0[:],
                                     start=True, stop=False)
                    nc.tensor.matmul(hp[:], xtb[:, r0:r0 + 128], rhsA1[:],
                                     start=False, stop=True)
                    ht = pa_ht.tile([128, 262], dt.float16, tag="ht")
                    if i % 2 == 0:
                        nc.scalar.copy(out=ht[:], in_=hp[:])
                    else:
                        nc.vector.tensor_copy(out=ht[:], in_=hp[:])
                    nc.gpsimd.memset(ht[:, 128:129], 1.0)
                    nc.gpsimd.memset(ht[:, 257:258], 1.0)
                    nc.sync.dma_start(out=htable[r0:r0 + 128, 0:262], in_=ht[:])

            if phases == "A":
                dbg = top.tile([128, 262], dt.float16, tag="dbgA")
                for i in range(RB):
                    nc.sync.dma_start(out=dbg[:], in_=htable[i * 128:(i + 1) * 128, 0:262])
                    nc.sync.dma_start(out=outD[i * 128:(i + 1) * 128, 0:262], in_=dbg[:])
                raise _PhaseDone()

            # ---- phase B: GAT aggregation per dst block ----
            xg_pool = est.enter_context(tc.tile_pool(name="xg", bufs=1))
            xgs = []
            with tc.tile_pool(name="pb_idx", bufs=2) as pb_idx, \
                 tc.tile_pool(name="pb_oht", bufs=3) as pb_oht, \
                 tc.tile_pool(name="pb_g", bufs=8) as pb_g, \
                 tc.tile_pool(name="pb_ad", bufs=2) as pb_ad, \
                 tc.tile_pool(name="pb_ex", bufs=4) as pb_ex, \
                 tc.tile_pool(name="pb_rhs", bufs=6) as pb_rhs, \
                 tc.tile_pool(name="pb_ep", bufs=2) as pb_ep, \
                 tc.tile_pool(name="pb_ps", bufs=2, space="PSUM") as pb_ps, \
                 tc.tile_pool(name="pb_adp", bufs=4, space="PSUM") as pb_adp:
                for _slot in range(8):
                    gz = pb_g.tile([128, GC, HROW], dt.float16, tag="g")
                    nc.vector.memset(gz[:], 0.0)
                for b in range(RB):
                    hix = pb_idx.tile([128, TB], dt.int32, tag="hix")
                    nc.sync.dma_start(out=hix[:], in_=hidxI[b])
                    oh_sb = pb_oht.tile([128, TB * 128], dt.float16, tag="oh_sb")
                    nc.sync.dma_start(out=oh_sb[:], in_=ohtab[b])
                    ohT_sb = pb_oht.tile([128, TB * 128], dt.float16, tag="ohT_sb")
                    nc.sync.dma_start(out=ohT_sb[:], in_=ohTtab[b])

                    adblk = pb_ad.tile([128, 2], dt.float16, tag="adblk")
                    ps = pb_ps.tile([128, 258], dt.float32, tag="ps", space="PSUM")

                    for c in range(NCH):
                        g = pb_g.tile([128, GC, HROW], dt.float16, tag="g")
                        for tl in range(GC):
                            nc.gpsimd.indirect_dma_start(
                                out=g[:, tl, :], out_offset=None,
                                in_=htable[:],
                                in_offset=bass.IndirectOffsetOnAxis(
                                    ap=hix[:, c * GC + tl:c * GC + tl + 1], axis=0))
                        if c == 0:
                            # tile 0 rows are this block's own dst rows (self loops)
                            nc.vector.tensor_copy(out=adblk[:], in_=g[:, 0, 260:262])
                        for hs in range(GC // SUB):
                            t0 = c * GC + hs * SUB
                            adps = pb_adp.tile([128, SUB, 2], dt.float32, tag="adps",
                                               space="PSUM")
                            for k in range(SUB):
                                t = t0 + k
                                nc.tensor.matmul(adps[:, k, :],
                                                 ohT_sb[:, t * 128:(t + 1) * 128],
                                                 adblk[:], start=True, stop=True)
                            # e = a_s[src] + a_d[dst]; exv = max(exp(e), exp(0.2e))
                            exr = pb_ex.tile([128, SUB, 2], dt.float32, tag="exr")
                            nc.vector.tensor_tensor(
                                out=exr[:], in0=adps[:],
                                in1=g[:, hs * SUB:(hs + 1) * SUB, 258:260], op=OP.add)
                            exn = pb_ex.tile([128, SUB, 2], dt.float32, tag="exn")
                            nc.scalar.activation(out=exn[:], in_=exr[:], func=AF.Exp,
                                                 scale=0.2)
                            exp_ = pb_ex.tile([128, SUB, 2], dt.float32, tag="exp_")
                            nc.scalar.activation(out=exp_[:], in_=exr[:], func=AF.Exp)
                            exv = pb_ex.tile([128, SUB, 2], dt.float32, tag="exv")
                            nc.vector.tensor_tensor(out=exv[:], in0=exp_[:], in1=exn[:],
                                                    op=OP.max)
                            for k in range(SUB):
                                t = t0 + k
                                tl = hs * SUB + k
                                rhs = pb_rhs.tile([128, 258], dt.float16, tag="rhs")
                                if t % 2 == 0:
                                    nc.scalar.activation(out=rhs[:, 0:129],
                                                         in_=g[:, tl, 0:129],
                                                         func=AF.Copy,
                                                         scale=exv[:, k, 0:1])
                                    nc.vector.tensor_scalar(
                                        out=rhs[:, 129:258], in0=g[:, tl, 129:258],
                                        scalar1=exv[:, k, 1:2], scalar2=None, op0=OP.mult)
                                else:
                                    nc.vector.tensor_scalar(
                                        out=rhs[:, 0:129], in0=g[:, tl, 0:129],
                                        scalar1=exv[:, k, 0:1], scalar2=None, op0=OP.mult)
                                    nc.scalar.activation(out=rhs[:, 129:258],
                                                         in_=g[:, tl, 129:258],
                                                         func=AF.Copy,
                                                         scale=exv[:, k, 1:2])
                                nc.tensor.matmul(ps[:],
                                                 oh_sb[:, t * 128:(t + 1) * 128],
                                                 rhs[:],
                                                 start=(t == 0), stop=(t == TB - 1))

                    # epilogue: normalize, +b_gat, lrelu(0.01) -> fp16 xg
                    rec0 = pb_ep.tile([128, 1], dt.float32, tag="rec0")
                    nc.vector.reciprocal(rec0[:], ps[:, 128:129])
                    rec1 = pb_ep.tile([128, 1], dt.float32, tag="rec1")
                    nc.vector.reciprocal(rec1[:], ps[:, 257:258])
                    xg = xg_pool.tile([128, 256], dt.float16, tag=f"xg{b}")
                    nc.scalar.activation(out=xg[:, 0:128], in_=ps[:, 0:128],
                                         func=AF.Copy, scale=rec0[:])
                    nc.scalar.activation(out=xg[:, 128:256], in_=ps[:, 129:257],
                                         func=AF.Copy, scale=rec1[:])
                    nc.vector.tensor_tensor(out=xg[:], in0=xg[:], in1=bgat_sb[:], op=OP.add)
                    ng = pb_ep.tile([128, 256], dt.float16, tag="ng")
                    nc.vector.tensor_scalar(out=ng[:], in0=xg[:], scalar1=0.0,
                                            scalar2=0.01, op0=OP.min, op1=OP.mult)
                    nc.vector.scalar_tensor_tensor(out=xg[:], in0=xg[:], scalar=0.0,
                                                   in1=ng[:], op0=OP.max, op1=OP.add)
                    xgs.append(xg)

            if phases == "AB":
                for b in range(RB):
                    nc.sync.dma_start(out=outD[b * 128:(b + 1) * 128, 0:256], in_=xgs[b][:])
                raise _PhaseDone()

            # ---- phase C: dense + LN on own shard (stage-parallel across blocks) ----
            cc_sb = top.tile([4, RPAD], dt.float32, tag="cc_sb")
            with tc.tile_pool(name="pc", bufs=12) as pc, \
                 tc.tile_pool(name="pc_ps", bufs=2, space="PSUM") as pc_ps, \
                 tc.tile_pool(name="pc_mm", bufs=4, space="PSUM") as pc_mm:

                def transpose16(xin, pdim, fdim):
                    # xin fp16 [pdim, fdim] -> sbuf fp16 [fdim, pdim]
                    p = pc_ps.tile([128, 128], dt.float16, tag="tpp16", space="PSUM")
                    nc.tensor.transpose(out=p[:fdim, 0:pdim], in_=xin,
                                        identity=ident16[:pdim, :pdim])
                    s = pc.tile([fdim, pdim], dt.float16, tag=f"tt{fdim}_{pdim}")
                    nc.scalar.copy(out=s[:], in_=p[:fdim, 0:pdim])
                    return s

                def c_chain(b):
                    x0 = xgs[b]
                    xt0 = transpose16(x0[:, 0:128], 128, 128)
                    yield
                    xt1 = transpose16(x0[:, 128:256], 128, 128)
                    yield
                    pA = pc_mm.tile([128, 128], dt.float32, tag="mm", space="PSUM")
                    nc.tensor.matmul(pA[:], xt0[:], waT0[:], start=True, stop=False)
                    nc.tensor.matmul(pA[:], xt1[:], waT1[:], start=False, stop=True)
                    yield
                    x1 = yield from ln_lrelu(b, pA[:], 128, ba_sb, lnaw_sb, lnab_sb)
                    x1t = transpose16(x1[:], 128, 128)
                    yield
                    p1 = pc_mm.tile([128, 64], dt.float32, tag="mm", space="PSUM")
                    nc.tensor.matmul(p1[:], x1t[:], w1T_sb[:], start=True, stop=True)
                    yield
                    x2 = yield from ln_lrelu(b, p1[:], 64, b1_sb, ln1w_sb, ln1b_sb)
                    x2t = transpose16(x2[:], 128, 64)
                    yield
                    p2 = pc_mm.tile([128, 32], dt.float32, tag="mm", space="PSUM")
                    nc.tensor.matmul(p2[:], x2t[:], w2T_sb[:], start=True, stop=True)
                    yield
                    x3 = yield from ln_lrelu(b, p2[:], 32, b2_sb, ln2w_sb, ln2b_sb)
                    x3t = transpose16(x3[:], 128, 32)
                    yield
                    p3 = pc_mm.tile([128, 3], dt.float32, tag="mm", space="PSUM")
                    nc.tensor.matmul(p3[:], x3t[:], w3T_sb[:], start=True, stop=True)
                    yield
                    y3 = pc.tile([128, 4], dt.float32, tag="y3")
                    nc.vector.tensor_tensor(out=y3[:, 0:3], in0=p3[:], in1=b3_sb[:], op=OP.add)
                    scr3 = pc.tile([128, 3], dt.float32, tag="scr3")
                    nc.scalar.activation(out=scr3[:], in_=y3[:, 0:3], func=AF.Square,
                                         accum_out=y3[:, 3:4])
                    yield
                    h3p = pc_ps.tile([128, 128], dt.float32, tag="tpp", space="PSUM")
                    nc.tensor.transpose(out=h3p[:4, 0:128], in_=y3[:], identity=ident[:])
                    nc.scalar.copy(out=cc_sb[:, b * 128:(b + 1) * 128], in_=h3p[:4, 0:128])

                def ln_lrelu(b, xin, fdim, bias_bc, w_bc, b_bc):
                    # y = xin + bias; u = LN(y)*w + b; return lrelu001(u) fp16
                    y = pc.tile([128, fdim], dt.float32, tag=f"y{fdim}")
                    nc.vector.tensor_tensor(out=y[:], in0=xin, in1=bias_bc[:], op=OP.add)
                    yield
                    scr = pc.tile([128, fdim], dt.float32, tag=f"scr{fdim}")
                    msum = pc.tile([128, 1], dt.float32, tag="msum")
                    nc.scalar.activation(out=scr[:], in_=y[:], func=AF.Copy,
                                         accum_out=msum[:])
                    sqs = pc.tile([128, 1], dt.float32, tag="sqs")
                    nc.scalar.activation(out=scr[:], in_=y[:], func=AF.Square,
                                         accum_out=sqs[:])
                    yield
                    mean = pc.tile([128, 1], dt.float32, tag="mean")
                    nc.vector.tensor_scalar(out=mean[:], in0=msum[:], scalar1=1.0 / fdim,
                                            scalar2=None, op0=OP.mult)
                    var = pc.tile([128, 1], dt.float32, tag="var")
                    nc.vector.tensor_scalar(out=var[:], in0=sqs[:], scalar1=1.0 / fdim,
                                            scalar2=None, op0=OP.mult)
                    m2 = pc.tile([128, 1], dt.float32, tag="m2")
                    nc.vector.tensor_scalar(out=m2[:], in0=mean[:], scalar1=mean[:, 0:1],
                                            scalar2=None, op0=OP.mult)
                    nc.vector.tensor_tensor(out=var[:], in0=var[:], in1=m2[:], op=OP.subtract)
                    sd = pc.tile([128, 1], dt.float32, tag="sd")
                    nc.scalar.activation(out=sd[:], in_=var[:], func=AF.Sqrt,
                                         bias=eps_col[:, 0:1])
                    rstd = pc.tile([128, 1], dt.float32, tag="rstd")
                    nc.vector.reciprocal(rstd[:], sd[:])
                    yield
                    u = pc.tile([128, fdim], dt.float32, tag=f"u{fdim}")
                    nc.vector.scalar_tensor_tensor(out=u[:], in0=y[:], scalar=mean[:, 0:1],
                                                   in1=w_bc[:], op0=OP.subtract, op1=OP.mult)
                    nc.vector.scalar_tensor_tensor(out=u[:], in0=u[:], scalar=rstd[:, 0:1],
                                                   in1=b_bc[:], op0=OP.mult, op1=OP.add)
                    yield
                    ngt = pc.tile([128, fdim], dt.float32, tag=f"ng{fdim}")
                    nc.vector.tensor_scalar(out=ngt[:], in0=u[:], scalar1=0.0,
                                            scalar2=0.01, op0=OP.min, op1=OP.mult)
                    u16 = pc.tile([128, fdim], dt.float16, tag=f"u16_{fdim}")
                    nc.vector.scalar_tensor_tensor(out=u16[:], in0=u[:], scalar=0.0,
                                                   in1=ngt[:], op0=OP.max, op1=OP.add)
                    yield
                    return u16

                gens = [c_chain(b) for b in range(RB)]

                def run_gens(idxs):
                    done = {b: False for b in idxs}
                    while not all(done.values()):
                        for b in idxs:
                            if not done[b]:
                                try:
                                    next(gens[b])
                                except StopIteration:
                                    done[b] = True

                run_gens(range(RB // 2))
                nc.sync.dma_start(out=cc_inA[:], in_=cc_sb[:, 0:HALF])
                nc.gpsimd.collective_compute(
                    "AllGather", OP.bypass, replica_groups=[list(range(CORES))],
                    ins=[cc_inA[:].opt()], outs=[cc_outA[:].opt()])
                run_gens(range(RB // 2, RB))
                nc.sync.dma_start(out=cc_inB[:], in_=cc_sb[:, HALF:RPAD])
                nc.gpsimd.collective_compute(
                    "AllGather", OP.bypass, replica_groups=[list(range(CORES))],
                    ins=[cc_inB[:].opt()], outs=[cc_outB[:].opt()])

            if phases == "ABC":
                dbg16 = top.tile([4, RPAD], dt.float16, tag="dbgc")
                nc.vector.tensor_copy(out=dbg16[:], in_=cc_sb[:])
                nc.sync.dma_start(out=outD[0:4, 0:RPAD], in_=dbg16[:])
                raise _PhaseDone()

            # ---- phase D: build split-fp16 cdist operands ----
            # lhsT16 rows: [-2a(3) | -2a(3) | -2b(3) | sqhi | sqlo | 1 | 1]
            # (compute in partition-0 tiles, assemble via sbuf-to-sbuf DMA)
            lhsT16 = top.tile([13, RPAD], dt.float16, tag="lhsT16")
            pd = est.enter_context(tc.tile_pool(name="pd", bufs=1))
            a_own = pd.tile([4, RPAD], dt.float16, tag="a_own")
            nc.vector.tensor_copy(out=a_own[:], in_=cc_sb[:])
            b_own = pd.tile([4, RPAD], dt.float16, tag="b_own")
            nc.vector.tensor_tensor(out=b_own[:], in0=cc_sb[:], in1=a_own[:],
                                    op=OP.subtract)
            na4 = pd.tile([4, RPAD], dt.float16, tag="na4")
            nc.scalar.activation(out=na4[:], in_=a_own[:], func=AF.Copy, scale=-2.0)
            nb4 = pd.tile([4, RPAD], dt.float16, tag="nb4")
            nc.scalar.activation(out=nb4[:], in_=b_own[:], func=AF.Copy, scale=-2.0)
            ones_r = pd.tile([2, RPAD], dt.float16, tag="ones_r")
            nc.vector.memset(ones_r[:], 1.0)
            nc.sync.dma_start(out=lhsT16[0:3, :], in_=na4[0:3, :])
            nc.sync.dma_start(out=lhsT16[3:6, :], in_=na4[0:3, :])
            nc.sync.dma_start(out=lhsT16[6:9, :], in_=nb4[0:3, :])
            nc.sync.dma_start(out=lhsT16[9:10, :], in_=a_own[3:4, :])
            nc.sync.dma_start(out=lhsT16[10:11, :], in_=b_own[3:4, :])
            nc.sync.dma_start(out=lhsT16[11:13, :], in_=ones_r[:])

            # rhs16 rows: [a(3) | b(3) | a(3) | 1 | 1 | sqhi | sqlo]
            rhs_f = pd.tile([4, NCOL], dt.float32, tag="rhs_f")
            nc.vector.memset(rhs_f[:, N:NCOL], 0.0)
            a16 = pd.tile([4, NCOL], dt.float16, tag="a16")
            b16 = pd.tile([4, NCOL], dt.float16, tag="b16")
            vA = lambda t, r0, r1: t[r0:r1, 0:10000].rearrange(
                "p (s c) -> p s c", s=8)[:, :, 0:HALF]
            vB = lambda t, r0, r1: t[r0:r1, 0:10000].rearrange(
                "p (s c) -> p s c", s=8)[:, :, HALF:SHARD]
            # half A (ready while collective B still running)
            for s in range(CORES):
                c0 = s * SHARD
                nc.sync.dma_start(out=rhs_f[0:4, c0:c0 + HALF],
                                  in_=cc_outA[:][s])
            nc.vector.tensor_copy(out=vA(a16, 0, 4), in_=vA(rhs_f, 0, 4))
            nc.vector.tensor_tensor(out=vA(b16, 0, 4), in0=vA(rhs_f, 0, 4),
                                    in1=vA(a16, 0, 4), op=OP.subtract)
            # half B
            for s in range(CORES):
                c0 = s * SHARD
                nc.sync.dma_start(out=rhs_f[0:4, c0 + HALF:c0 + SHARD],
                                  in_=cc_outB[:][s, 0:4, 0:SHARD - HALF])
            nc.vector.tensor_copy(out=vB(a16, 0, 4), in_=vB(rhs_f, 0, 4))
            nc.vector.tensor_tensor(out=vB(b16, 0, 4), in0=vB(rhs_f, 0, 4),
                                    in1=vB(a16, 0, 4), op=OP.subtract)
            # pad cols 10000:10240
            nc.scalar.copy(out=a16[:, N:NCOL], in_=rhs_f[:, N:NCOL])
            nc.scalar.copy(out=b16[:, N:NCOL], in_=rhs_f[:, N:NCOL])
            ones_n = pd.tile([2, NCOL], dt.float16, tag="ones_n")
            nc.vector.memset(ones_n[:], 1.0)
            rhs16 = top.tile([13, NCOL], dt.float16, tag="rhs16")
            nc.sync.dma_start(out=rhs16[0:3, :], in_=a16[0:3, :])
            nc.sync.dma_start(out=rhs16[3:6, :], in_=b16[0:3, :])
            nc.sync.dma_start(out=rhs16[6:9, :], in_=a16[0:3, :])
            nc.sync.dma_start(out=rhs16[9:11, :], in_=ones_n[:])
            nc.sync.dma_start(out=rhs16[11:12, :], in_=a16[3:4, :])
            nc.sync.dma_start(out=rhs16[12:13, :], in_=b16[3:4, :])

            if phases == "ABCD":
                nc.sync.dma_start(out=outD[0:13, 0:NCOL], in_=rhs16[:])
                raise _PhaseDone()

            # ---- phase E: cdist row-block x col-chunk (split-fp16 matmul, fp16 out) ----
            MRG = 5   # psum chunks merged into one output tile/DMA
            with tc.tile_pool(name="pe_d", bufs=3) as pe_d, \
                 tc.tile_pool(name="pe_d2", bufs=3) as pe_d2, \
                 tc.tile_pool(name="pe_ps", bufs=6, space="PSUM") as pe_ps:
                for rb in range(RB):
                    for mg in range(NCOL // (CCH * MRG)):
                        d2t = pe_d2.tile([128, CCH * MRG], dt.float16, tag="d2t")
                        for k in range(MRG):
                            ch = mg * MRG + k
                            dp = pe_ps.tile([128, CCH], dt.float32, tag="dp", space="PSUM")
                            nc.tensor.matmul(
                                dp[:], lhsT16[:, rb * 128:(rb + 1) * 128],
                                rhs16[:, ch * CCH:(ch + 1) * CCH],
                                start=True, stop=True)
                            nc.scalar.activation(out=d2t[:, k * CCH:(k + 1) * CCH],
                                                 in_=dp[:], func=AF.Sqrt,
                                                 bias=eps4_col[:, 0:1])
                        nc.sync.dma_start(
                            out=outD[rb * 128:(rb + 1) * 128,
                                     mg * CCH * MRG:(mg + 1) * CCH * MRG],
                            in_=d2t[:])

    except _PhaseDone:
        pass
    _tc_cm.__exit__(None, None, None)
    nc.compile()
    return nc


def _prep_host(x, edge_index):
    xp = np.zeros((NPAD, FIN), np.float32)
    xp[:N] = np.asarray(x, np.float32)
    xp16 = np.ascontiguousarray(xp.T.astype(np.float16))  # [256, NPAD]

    ei = np.asarray(edge_index)
    src = ei[0].astype(np.int64)
    dst = ei[1].astype(np.int64)

    core = dst // SHARD
    per_core = []
    max_tiles = 0
    for c in range(CORES):
        sel = core == c
        s_c = src[sel]
        d_c = dst[sel]
        loc = d_c - c * SHARD
        blk = loc // 128
        dl = loc - blk * 128
        blocks = []
        for b in range(RB):
            m = blk == b
            blocks.append((s_c[m], dl[m]))
            # tile 0 holds the block's self-loop edges; rest start at tile 1
            max_tiles = max(max_tiles, 1 + (len(blocks[-1][0]) + 127) // 128)
        per_core.append(blocks)

    TB = GC * ((max_tiles + GC - 1) // GC)
    S16 = 8 * TB
    NE = TB * 128

    hidx = np.zeros((CORES, RB, 128, TB), np.int32)
    ohtab = np.zeros((CORES, RB, 128, NE), np.float16)
    ohTtab = np.zeros((CORES, RB, 128, NE), np.float16)
    for c in range(CORES):
        for b in range(RB):
            # tile 0: self loops (edge at partition p has src=dst=block row p)
            rows = c * SHARD + b * 128 + np.arange(128)
            real = rows < N
            crows = np.minimum(rows, N - 1)
            jr = np.arange(128)
            hidx[c, b, jr, 0] = crows.astype(np.int32)
            pr = jr[real]
            ohtab[c, b, pr, pr] = 1.0
            ohTtab[c, b, pr, pr] = 1.0
            # remaining edges from tile 1 on
            s_b, dl_b = per_core[c][b]
            n = len(s_b)
            js = 128 + np.arange(n)
            p = js % 128          # edge partition
            t = js // 128         # edge tile (>= 1)
            hidx[c, b, p, t] = s_b.astype(np.int32)
            sl = dl_b.astype(np.int64)
            ohtab[c, b, p, t * 128 + sl] = 1.0
            ohTtab[c, b, sl, t * 128 + p] = 1.0
    return xp16, hidx, ohtab, ohTtab, TB


def build_in_maps(inputs):
    xp16, hidx, ohtab, ohTtab, TB = _prep_host(inputs["x"], inputs["edge_index"])

    def bc(vec, n, f16=False):
        v = np.asarray(vec, np.float32).reshape(1, n)
        out = np.ascontiguousarray(np.broadcast_to(v, (128, n)).copy())
        return out.astype(np.float16) if f16 else out

    # rhsA: [256 (xfeat, 2 chunks of 128), 262] fp16
    # cols: 0:128 WgT head0 | 128 zero | 129:257 WgT head1 | 257 zero | 258:262 wtil
    Wg = np.asarray(inputs["W_gat"], np.float32)       # [256, 256] rows = H*F out
    att_src = np.asarray(inputs["att_src"], np.float32)  # [2, 128]
    att_dst = np.asarray(inputs["att_dst"], np.float32)
    rhsA = np.zeros((256, 262), np.float32)
    rhsA[:, 0:128] = Wg[0:128, :].T
    rhsA[:, 129:257] = Wg[128:256, :].T
    rhsA[:, 258] = Wg[0:128, :].T @ att_src[0]
    rhsA[:, 259] = Wg[128:256, :].T @ att_src[1]
    rhsA[:, 260] = Wg[0:128, :].T @ att_dst[0]
    rhsA[:, 261] = Wg[128:256, :].T @ att_dst[1]
    rhsA16 = rhsA.astype(np.float16).reshape(2, 128, 262)

    Wa = np.asarray(inputs["Wa"], np.float32)  # [128, 256]
    waT16 = np.ascontiguousarray(Wa.T.astype(np.float16)).reshape(2, 128, 128)
    w1T16 = np.ascontiguousarray(np.asarray(inputs["W1"], np.float32).T.astype(np.float16))
    w2T16 = np.ascontiguousarray(np.asarray(inputs["W2"], np.float32).T.astype(np.float16))
    w3T16 = np.ascontiguousarray(np.asarray(inputs["W3"], np.float32).T.astype(np.float16))

    shared = {
        "xt16": xp16,
        "rhsA": np.ascontiguousarray(rhsA16),
        "waT": waT16, "w1T": w1T16, "w2T": w2T16, "w3T": w3T16,
        "bgat_bc": bc(inputs["b_gat"], 256, f16=True),
        "ba_bc": bc(inputs["ba"], 128),
        "lnaw_bc": bc(inputs["lna_w"], 128), "lnab_bc": bc(inputs["lna_b"], 128),
        "b1_bc": bc(inputs["b1"], 64),
        "ln1w_bc": bc(inputs["ln1_w"], 64), "ln1b_bc": bc(inputs["ln1_b"], 64),
        "b2_bc": bc(inputs["b2"], 32),
        "ln2w_bc": bc(inputs["ln2_w"], 32), "ln2b_bc": bc(inputs["ln2_b"], 32),
        "b3_bc": bc(inputs["b3"], 3),
    }
    in_maps = [
        {**shared, "hidxI": np.ascontiguousarray(hidx[c]),
         "ohtab": np.ascontiguousarray(ohtab[c]),
         "ohTtab": np.ascontiguousarray(ohTtab[c])}
        for c in range(CORES)
    ]
    return in_maps, TB


def kernel(**inputs):
    in_maps, TB = build_in_maps(inputs)

    import os
    phases = os.environ.get("K_PHASES", "ABCDE")
    key = (TB, phases)
    if key not in _BUILD_CACHE:
        _BUILD_CACHE[key] = _build(TB, phases)
    nc = _BUILD_CACHE[key]
    res = run_bass_kernel_spmd(nc, in_maps, core_ids=list(range(CORES)))
    global _LAST_RESULTS
    _LAST_RESULTS = res.results
    out = np.empty((N, N), np.float32)
    for c in range(CORES):
        out[c * SHARD:(c + 1) * SHARD, :] = \
            res.results[c]["outD"][:SHARD, :N].astype(np.float32)
    return out
